# revision 2
# baseline (speedup 1.0000x reference)
"""HRNN Trainium2 kernel: Jacobi trajectory iteration for the recurrence.

Algorithm: the tanh-RNN recurrence h_t = tanh(Wh h_{t-1} + u_t) is solved per
layer by full-trajectory Jacobi fixed-point sweeps H^{m+1} = tanh(Wh H^m + U)
(14 sweeps) instead of T=128 sequential steps.  Each sweep is a batched matmul
over all T timesteps, amortizing PE weight loads 128-way; the sequential
formulation reloads 32 weight tiles per step and is LDWEIGHTS-bound (~650us).
Convergence is geometric (~0.55x/sweep, tanh saturation); 14 sweeps -> ~9e-3
end-to-end rel err (validated vs the exact reference on the real weights).

Sharding: expert-parallel, 2 encoders per core over 8 cores.  The decoder is
row(k)-sharded: each core computes full-width partial preactivations from its
local shard, combined with ReduceScatter(add) at the three layer boundaries;
the output layer's partials go back per-core and the HOST sums 8 partials.
Biases enter matmul-side via K=1 row-matmuls against a ones vector (b/8 per
core, summed by the collectives).  Everything is fp16 on the PE (same speed
as bf16, 8x lower quantization noise); PSUM accumulation fp32.

Schedule highlights:
- wff1/wff2 prefetched during the Jacobi sweeps (ff pool co-resident with the
  rnn pool; was: ff DMA stalled the FF phase ~35us).
- j-granular DVE-add + tanh in the sweep loop (act_j overlaps the next
  j-group's matmuls; cuts the per-sweep critical path ~0.8us).
- decoder weights staged late: wd0 into the freed rnn space during FF,
  wdm/wdo after the ff pool closes (hidden behind ReduceScatter 0).
- collective payloads fp16 (half the wire bytes of f32).
- single rotating PSUM pool for FF/decoder accumulation groups.

See kernel_v2.py docstring for the algorithm description.
"""

import sys
import numpy as np

sys.path.insert(0, "/opt/trn_rl_repo")

import ml_dtypes

E = 16
L = 3
D_IN = 32
D = 512
H_FF = 2048
D_ENC = 512
N_DEC = 4
H_DEC = 2048
D_OUT = 1024
T_FULL = 128
N_CORES = 8

E_LOC = E // N_CORES
DT = D // 128
NFT = H_FF // 128
NCAT = (L * D) // 128
NHD = H_DEC // 128
KD0 = (E_LOC * D_ENC) // 128
HD_SH = H_DEC // N_CORES
KDM = HD_SH // 128
DO_SH = D_OUT // N_CORES

S_SWEEPS = 14

F16 = ml_dtypes.float16 if hasattr(ml_dtypes, "float16") else np.float16


def _tile_kxm(w):
    """[K, M] -> [128, nk*nm*128] with col ((i*nm)+j)*128 : lhsT tile (i,j)."""
    K, M = w.shape
    nk, nm = K // 128, M // 128
    return np.ascontiguousarray(
        w.reshape(nk, 128, nm, 128).transpose(1, 0, 2, 3).reshape(128, nk * nm * 128)
    )


def _bias_cols(b):
    """[M] -> [128, M//128] with col j holding b[j*128:(j+1)*128]."""
    return np.ascontiguousarray(b.reshape(-1, 128).T)


def build_nc(t_steps):
    from concourse import bacc, mybir, tile

    F32 = mybir.dt.float32
    FP16 = mybir.dt.float16
    AF = mybir.ActivationFunctionType
    ADD = mybir.AluOpType.add
    T = t_steps

    nc = bacc.Bacc(None, num_devices=N_CORES)

    # ---- I/O ----------------------------------------------------------------
    xT = nc.dram_tensor("xT", [D_IN, T], F32, kind="ExternalInput")
    win0 = [nc.dram_tensor(f"win0_{k}", [D_IN, D], F32, kind="ExternalInput")
            for k in range(E_LOC)]
    wh = [nc.dram_tensor(f"wh_{k}", [128, L * DT * DT * 128], FP16, kind="ExternalInput")
          for k in range(E_LOC)]
    win = [nc.dram_tensor(f"win_{k}", [128, (L - 1) * DT * DT * 128], FP16,
                          kind="ExternalInput") for k in range(E_LOC)]
    b_rnn = [nc.dram_tensor(f"b_{k}", [128, L * DT], F32, kind="ExternalInput")
             for k in range(E_LOC)]
    wff1 = [nc.dram_tensor(f"wff1_{k}", [128, NCAT * NFT * 128], FP16, kind="ExternalInput")
            for k in range(E_LOC)]
    bff1 = [nc.dram_tensor(f"bff1_{k}", [128, NFT], F32, kind="ExternalInput")
            for k in range(E_LOC)]
    wff2 = [nc.dram_tensor(f"wff2_{k}", [128, NFT * DT * 128], FP16, kind="ExternalInput")
            for k in range(E_LOC)]
    bff2 = [nc.dram_tensor(f"bff2_{k}", [128, DT], F32, kind="ExternalInput")
            for k in range(E_LOC)]
    wd0 = nc.dram_tensor("wd0", [128, KD0 * NHD * 128], FP16, kind="ExternalInput")
    bd0r = nc.dram_tensor("bd0r", [1, NHD * 128], FP16, kind="ExternalInput")
    wdm = [nc.dram_tensor(f"wdm{m}", [128, KDM * NHD * 128], FP16, kind="ExternalInput")
           for m in range(N_DEC - 2)]
    bdmr = [nc.dram_tensor(f"bdmr{m}", [1, NHD * 128], FP16, kind="ExternalInput")
            for m in range(N_DEC - 2)]
    NDO = D_OUT // 128            # 8 output j-tiles
    wdo = nc.dram_tensor("wdo", [128, KDM * NDO * 128], FP16, kind="ExternalInput")
    bdor = nc.dram_tensor("bdor", [1, NDO * 128], FP16, kind="ExternalInput")
    # per-core output-layer PARTIAL; the host sums the 8 cores' tensors
    y_out = nc.dram_tensor("y_out", [D_OUT, T], F32, kind="ExternalOutput")

    cc_in = [nc.dram_tensor(f"cc_in{m}", [H_DEC, T], FP16) for m in range(N_DEC - 1)]
    rs_out = [nc.dram_tensor(f"rs_out{m}", [HD_SH, T], FP16) for m in range(N_DEC - 1)]

    RG = [list(range(N_CORES))]

    def colw(i, j, nm):
        return (i * nm + j) * 128

    LCH = DT * DT * 128

    with tile.TileContext(nc, num_cores=N_CORES) as tc:
        with (
            tc.tile_pool(name="persist", bufs=1) as persist,
            tc.tile_pool(name="ps_main", bufs=4, space="PSUM") as ps_main,
            tc.tile_pool(name="tmp", bufs=4) as tmp_pool,
        ):
            # --- small persistent tensors
            xT_sb = persist.tile([D_IN, T], F32, name="xT", tag="xT")
            nc.sync.dma_start(xT_sb[:], xT[:])
            win0_sb, b_sb, bff1_sb, bff2_sb = [], [], [], []
            u_sb, ench_sb = [], []
            hh = [[None] * L for _ in range(E_LOC)]
            for k in range(E_LOC):
                w0 = persist.tile([D_IN, D], F32, name=f"win0_{k}", tag=f"win0_{k}")
                nc.sync.dma_start(w0[:], win0[k][:])
                win0_sb.append(w0)
                bb = persist.tile([128, L * DT], F32, name=f"b_{k}", tag=f"b_{k}")
                nc.sync.dma_start(bb[:], b_rnn[k][:])
                b_sb.append(bb)
                b1 = persist.tile([128, NFT], F32, name=f"bff1_{k}", tag=f"bff1_{k}")
                nc.sync.dma_start(b1[:], bff1[k][:])
                bff1_sb.append(b1)
                b2 = persist.tile([128, DT], F32, name=f"bff2_{k}", tag=f"bff2_{k}")
                nc.sync.dma_start(b2[:], bff2[k][:])
                bff2_sb.append(b2)
                u_sb.append(persist.tile([128, DT, T], FP16, name=f"u_{k}", tag=f"u_{k}"))
                ench_sb.append(persist.tile([128, DT, T], FP16, name=f"enc_{k}", tag=f"enc_{k}"))
                for l in range(L):
                    hh[k][l] = [
                        persist.tile([128, DT, 1 + T], FP16, name=f"hh_{k}_{l}_{b}",
                                     tag=f"hh_{k}_{l}_{b}")
                        for b in range(2)
                    ]
                    nc.vector.memset(hh[k][l][0][:], 0.0)
                    nc.vector.memset(hh[k][l][1][:, :, 0:1], 0.0)
            ones_sb = persist.tile([1, T], FP16, name="ones", tag="ones")
            nc.vector.memset(ones_sb[:], 1.0)
            bd0r_sb = persist.tile([1, NHD * 128], FP16, name="bd0r", tag="bd0r")
            nc.sync.dma_start(bd0r_sb[:], bd0r[:])

            with tc.tile_pool(name="ff", bufs=1) as ff:
                wff1_sb, wff2_sb, ffs_sb = [], [], []
                ffacc_sb = []
                for k in range(E_LOC):
                    wff1_sb.append(ff.tile([128, NCAT * NFT * 128], FP16,
                                           name=f"wff1_{k}", tag=f"wff1_{k}"))
                    wff2_sb.append(ff.tile([128, NFT * DT * 128], FP16,
                                           name=f"wff2_{k}", tag=f"wff2_{k}"))
                    ffacc_sb.append(ff.tile([128, NFT, T], FP16,
                                            name=f"ffacc_{k}", tag=f"ffacc_{k}"))

                with (
                    tc.tile_pool(name="rnn", bufs=1) as rnn,
                    tc.tile_pool(name="ps_sw", bufs=2, space="PSUM") as ps_sw,
                ):
                    wh_sb, win_sb = [], []
                    for k in range(E_LOC):
                        wh_sb.append(rnn.tile([128, L * LCH], FP16, name=f"wh_{k}", tag=f"wh_{k}"))
                        win_sb.append(rnn.tile([128, (L - 1) * LCH], FP16, name=f"win_{k}",
                                               tag=f"win_{k}"))
                    for l in range(L):
                        for k in range(E_LOC):
                            nc.sync.dma_start(wh_sb[k][:, l * LCH:(l + 1) * LCH],
                                              wh[k][:, l * LCH:(l + 1) * LCH])
                            if l < L - 1:
                                nc.sync.dma_start(win_sb[k][:, l * LCH:(l + 1) * LCH],
                                                  win[k][:, l * LCH:(l + 1) * LCH])
                    # ff weights stream in during the sweeps
                    for k in range(E_LOC):
                        half = NCAT * NFT * 128 // 2
                        nc.sync.dma_start(wff1_sb[k][:, 0:half], wff1[k][:, 0:half])
                        nc.sync.dma_start(wff1_sb[k][:, half:], wff1[k][:, half:])
                        nc.sync.dma_start(wff2_sb[k][:], wff2[k][:])

                    hfin = [[None] * E_LOC for _ in range(L)]
                    for l in range(L):
                        for k in range(E_LOC):
                            psu = ps_sw.tile([128, DT, T], F32, name=f"ps{k}", tag=f"ps{k}")
                            if l == 0:
                                for j in range(DT):
                                    nc.tensor.matmul(psu[:, j, :],
                                                     win0_sb[k][:, j * 128:(j + 1) * 128],
                                                     xT_sb[:], start=True, stop=True)
                            else:
                                hprev = hfin[l - 1][k]
                                for j in range(DT):
                                    for i in range(DT):
                                        nc.tensor.matmul(
                                            psu[:, j, :],
                                            win_sb[k][:, colw((l - 1) * DT + i, j, DT):
                                                      colw((l - 1) * DT + i, j, DT) + 128],
                                            hprev[:, i, 1:1 + T],
                                            start=(i == 0), stop=(i == DT - 1))
                            for j in range(DT):
                                nc.vector.tensor_scalar_add(
                                    u_sb[k][:, j, :], psu[:, j, :],
                                    b_sb[k][:, l * DT + j:l * DT + j + 1])

                        for s in range(S_SWEEPS):
                            for k in range(E_LOC):
                                src = hh[k][l][s % 2]
                                dst = hh[k][l][(s + 1) % 2]
                                if s == 0:
                                    # zero state: sweep 0 is just tanh(u)
                                    nc.scalar.activation(dst[:, :, 1:1 + T], u_sb[k][:],
                                                         AF.Tanh)
                                    continue
                                ps = ps_sw.tile([128, DT, T], F32, name=f"ps{k}", tag=f"ps{k}")
                                for jp in range(DT // 2):
                                    for j in (2 * jp, 2 * jp + 1):
                                        for i in range(DT):
                                            nc.tensor.matmul(
                                                ps[:, j, :],
                                                wh_sb[k][:, colw(l * DT + i, j, DT):
                                                         colw(l * DT + i, j, DT) + 128],
                                                src[:, i, 0:T],
                                                start=(i == 0), stop=(i == DT - 1))
                                    tt = tmp_pool.tile([128, 2, T], FP16, name=f"tt{k}",
                                                       tag=f"tt{k}")
                                    nc.vector.tensor_add(tt[:], ps[:, 2 * jp:2 * jp + 2, :],
                                                         u_sb[k][:, 2 * jp:2 * jp + 2, :])
                                    nc.scalar.activation(dst[:, 2 * jp:2 * jp + 2, 1:1 + T],
                                                         tt[:], AF.Tanh)
                        for k in range(E_LOC):
                            hfin[l][k] = hh[k][l][S_SWEEPS % 2]

                # --- FF head + decoder layer-0 partial (rnn space now free) --
                with tc.tile_pool(name="dec_w", bufs=1) as dec_w:
                    ffs_sb = ffacc_sb   # gelu output overwrites the staging buffer
                    pd_sb = dec_w.tile([128, NHD, T], FP16, name="pd_sb", tag="pd_sb")
                    wd0_sb = dec_w.tile([128, KD0 * NHD * 128], FP16, name="wd0", tag="wd0")
                    csz = KD0 * NHD * 128 // 4
                    for ch in range(4):
                        nc.sync.dma_start(wd0_sb[:, ch * csz:(ch + 1) * csz],
                                          wd0[:, ch * csz:(ch + 1) * csz])

                    for k in range(E_LOC):
                        for g in range(NFT // DT):          # 4 m-tiles per psum bank
                            pf = ps_main.tile([128, DT, T], F32, name="pm", tag="pm")
                            for mi in range(DT):
                                m = g * DT + mi
                                idx = 0
                                for l in range(L):
                                    for j in range(DT):
                                        nc.tensor.matmul(
                                            pf[:, mi, :],
                                            wff1_sb[k][:, colw(l * DT + j, m, NFT):
                                                       colw(l * DT + j, m, NFT) + 128],
                                            hfin[l][k][:, j, 1:1 + T],
                                            start=(idx == 0), stop=(idx == NCAT - 1))
                                        idx += 1
                                nc.scalar.activation(ffs_sb[k][:, m, :], pf[:, mi, :],
                                                     AF.Gelu_apprx_tanh,
                                                     bias=bff1_sb[k][:, m:m + 1])
                        pf2 = ps_main.tile([128, DT, T], F32, name="pm", tag="pm")
                        for j in range(DT):
                            for i in range(NFT):
                                nc.tensor.matmul(
                                    pf2[:, j, :],
                                    wff2_sb[k][:, colw(i, j, DT):colw(i, j, DT) + 128],
                                    ffs_sb[k][:, i, :],
                                    start=(i == 0), stop=(i == NFT - 1))
                            nc.vector.tensor_scalar_add(ench_sb[k][:, j, :], pf2[:, j, :],
                                                        bff2_sb[k][:, j:j + 1])

                    # decoder layer 0: k-sharded partial over this core's encoders
                    for g in range(NHD // DT):
                        pd = ps_main.tile([128, DT, T], F32, name="pm", tag="pm")
                        for ji in range(DT):
                            j2 = g * DT + ji
                            nc.tensor.matmul(pd[:, ji, :],
                                             bd0r_sb[:, j2 * 128:(j2 + 1) * 128],
                                             ones_sb[:], start=True, stop=False)
                            for i in range(KD0):
                                nc.tensor.matmul(
                                    pd[:, ji, :],
                                    wd0_sb[:, colw(i, j2, NHD):colw(i, j2, NHD) + 128],
                                    ench_sb[i // DT][:, i % DT, :],
                                    start=False, stop=(i == KD0 - 1))
                        nc.vector.tensor_copy(pd_sb[:, g * DT:(g + 1) * DT, :], pd[:])
                        nc.sync.dma_start(
                            cc_in[0][g * 512:(g + 1) * 512, :].rearrange(
                                "(i p) t -> p i t", p=128),
                            pd_sb[:, g * DT:(g + 1) * DT, :])
                    nc.gpsimd.collective_compute(
                        "ReduceScatter", ADD, replica_groups=RG,
                        ins=[cc_in[0][:]], outs=[rs_out[0][:]])

            # --- decoder mid/out (ff space now free) -------------------------
            with tc.tile_pool(name="dec2", bufs=1) as dec2:
                wdm_sb, bdmr_sb = [], []
                for m in range(N_DEC - 2):
                    t_ = dec2.tile([128, KDM * NHD * 128], FP16, name=f"wdm{m}", tag=f"wdm{m}")
                    nc.sync.dma_start(t_[:], wdm[m][:])
                    wdm_sb.append(t_)
                    t_ = dec2.tile([1, NHD * 128], FP16, name=f"bdmr{m}", tag=f"bdmr{m}")
                    nc.sync.dma_start(t_[:], bdmr[m][:])
                    bdmr_sb.append(t_)
                wdo_sb = dec2.tile([128, KDM * NDO * 128], FP16, name="wdo", tag="wdo")
                nc.sync.dma_start(wdo_sb[:], wdo[:])
                bdor_sb = dec2.tile([1, NDO * 128], FP16, name="bdor", tag="bdor")
                nc.sync.dma_start(bdor_sb[:], bdor[:])
                pd_sb = dec2.tile([128, NHD, T], FP16, name="pd_sb2", tag="pd_sb2")

                for m in range(N_DEC - 2):
                    zin = dec2.tile([128, KDM, T], FP16, name=f"zin{m}", tag=f"zin{m}")
                    nc.sync.dma_start(
                        zin[:], rs_out[m][:].rearrange("(i p) t -> p i t", p=128))
                    zloc = dec2.tile([128, KDM, T], FP16, name=f"z{m}", tag=f"z{m}")
                    nc.scalar.activation(zloc[:], zin[:], AF.Tanh)
                    pms = []
                    for g in range(NHD // DT):       # bias rows: no dep on the RS
                        pm = ps_main.tile([128, DT, T], F32, name="pm", tag="pm")
                        pms.append(pm)
                        for ji in range(DT):
                            j2 = g * DT + ji
                            nc.tensor.matmul(pm[:, ji, :],
                                             bdmr_sb[m][:, j2 * 128:(j2 + 1) * 128],
                                             ones_sb[:], start=True, stop=False)
                    for g in range(NHD // DT):
                        pm = pms[g]
                        for ji in range(DT):
                            j2 = g * DT + ji
                            for i in range(KDM):
                                nc.tensor.matmul(
                                    pm[:, ji, :],
                                    wdm_sb[m][:, colw(i, j2, NHD):colw(i, j2, NHD) + 128],
                                    zloc[:, i, :],
                                    start=False, stop=(i == KDM - 1))
                        nc.vector.tensor_copy(pd_sb[:, g * DT:(g + 1) * DT, :], pm[:])
                        nc.sync.dma_start(
                            cc_in[m + 1][g * 512:(g + 1) * 512, :].rearrange(
                                "(i p) t -> p i t", p=128),
                            pd_sb[:, g * DT:(g + 1) * DT, :])
                    nc.gpsimd.collective_compute(
                        "ReduceScatter", ADD, replica_groups=RG,
                        ins=[cc_in[m + 1][:]], outs=[rs_out[m + 1][:]])

                # output layer: k-sharded partial, ReduceScatter straight into y_out
                zin3 = dec2.tile([128, KDM, T], FP16, name="zin3", tag="zin3")
                nc.sync.dma_start(
                    zin3[:], rs_out[N_DEC - 2][:].rearrange("(i p) t -> p i t", p=128))
                z3 = dec2.tile([128, KDM, T], FP16, name="z3", tag="z3")
                nc.scalar.activation(z3[:], zin3[:], AF.Tanh)
                yp_sb = dec2.tile([128, NDO, T], F32, name="yp_sb", tag="yp_sb")
                pys = []
                for g in range(NDO // DT):
                    py = ps_main.tile([128, DT, T], F32, name="pm", tag="pm")
                    pys.append(py)
                    for ji in range(DT):
                        j2 = g * DT + ji
                        nc.tensor.matmul(py[:, ji, :],
                                         bdor_sb[:, j2 * 128:(j2 + 1) * 128],
                                         ones_sb[:], start=True, stop=False)
                for g in range(NDO // DT):
                    py = pys[g]
                    for ji in range(DT):
                        j2 = g * DT + ji
                        for i in range(KDM):
                            nc.tensor.matmul(
                                py[:, ji, :],
                                wdo_sb[:, colw(i, j2, NDO):colw(i, j2, NDO) + 128],
                                z3[:, i, :],
                                start=False, stop=(i == KDM - 1))
                    nc.vector.tensor_copy(yp_sb[:, g * DT:(g + 1) * DT, :], py[:])
                    nc.sync.dma_start(
                        y_out[g * 512:(g + 1) * 512, :].rearrange("(i p) t -> p i t", p=128),
                        yp_sb[:, g * DT:(g + 1) * DT, :])

    nc.compile()
    return nc


def prep_inputs(inputs, t_steps):
    """Build the 8 per-core input maps from full numpy inputs."""
    T = t_steps
    f32 = lambda a: np.asarray(a, np.float32)
    x = f32(inputs["x"])
    W_in0, Wh0, b0 = f32(inputs["W_in0"]), f32(inputs["Wh0"]), f32(inputs["b0"])
    W_in_rest, Wh_rest, b_rest = (f32(inputs["W_in_rest"]), f32(inputs["Wh_rest"]),
                                  f32(inputs["b_rest"]))
    W_ff1, b_ff1 = f32(inputs["W_ff1"]), f32(inputs["b_ff1"])
    W_ff2, b_ff2 = f32(inputs["W_ff2"]), f32(inputs["b_ff2"])
    W_d0, b_d0 = f32(inputs["W_d0"]), f32(inputs["b_d0"])
    W_dmid, b_dmid = f32(inputs["W_dmid"]), f32(inputs["b_dmid"])
    W_dout, b_dout = f32(inputs["W_dout"]), f32(inputs["b_dout"])

    xT = np.ascontiguousarray(x[0, :T].T)
    in_maps = []
    for c in range(N_CORES):
        m = {"xT": xT}
        for k in range(E_LOC):
            e = E_LOC * c + k
            m[f"win0_{k}"] = np.ascontiguousarray(W_in0[e])
            wh_all = np.concatenate([Wh0[e][None], Wh_rest[e]], 0)
            m[f"wh_{k}"] = _tile_kxm(wh_all.reshape(L * D, D)).astype(F16)
            m[f"win_{k}"] = _tile_kxm(W_in_rest[e].reshape((L - 1) * D, D)).astype(F16)
            b_all = np.concatenate([b0[e][None], b_rest[e]], 0).reshape(-1)
            m[f"b_{k}"] = _bias_cols(b_all)
            m[f"wff1_{k}"] = _tile_kxm(W_ff1[e]).astype(F16)
            m[f"bff1_{k}"] = _bias_cols(b_ff1[e])
            m[f"wff2_{k}"] = _tile_kxm(W_ff2[e]).astype(F16)
            m[f"bff2_{k}"] = _bias_cols(b_ff2[e])
        m["wd0"] = _tile_kxm(W_d0[c * E_LOC * D_ENC:(c + 1) * E_LOC * D_ENC, :]).astype(F16)
        m["bd0r"] = np.ascontiguousarray((b_d0 / N_CORES)[None, :]).astype(F16)
        for mm in range(N_DEC - 2):
            m[f"wdm{mm}"] = _tile_kxm(W_dmid[mm][c * HD_SH:(c + 1) * HD_SH, :]).astype(F16)
            m[f"bdmr{mm}"] = np.ascontiguousarray((b_dmid[mm] / N_CORES)[None, :]).astype(F16)
        m["wdo"] = _tile_kxm(W_dout[c * HD_SH:(c + 1) * HD_SH, :]).astype(F16)
        m["bdor"] = np.ascontiguousarray((b_dout / N_CORES)[None, :]).astype(F16)
        in_maps.append(m)
    return in_maps


def run(inputs, t_steps=T_FULL, trace=False):
    from concourse.bass_utils import run_bass_kernel_spmd

    nc = build_nc(t_steps)
    in_maps = prep_inputs(inputs, t_steps)
    res = run_bass_kernel_spmd(nc, in_maps, list(range(N_CORES)), trace=trace)
    acc = np.zeros((D_OUT, t_steps), np.float32)
    for c in range(N_CORES):
        acc += np.asarray(res.results[c]["y_out"], np.float32)
    return acc.T[None], res


def kernel(**inputs):
    y, _ = run(inputs, T_FULL, trace=False)
    return y


# revision 3
# speedup vs baseline: 1.0118x; 1.0118x over previous
"""HRNN Trainium2 kernel: Jacobi trajectory iteration for the recurrence.

Algorithm: the tanh-RNN recurrence h_t = tanh(Wh h_{t-1} + u_t) is solved per
layer by full-trajectory Jacobi fixed-point sweeps H^{m+1} = tanh(Wh H^m + U)
(14 sweeps) instead of T=128 sequential steps.  Each sweep is a batched matmul
over all T timesteps, amortizing PE weight loads 128-way; the sequential
formulation reloads 32 weight tiles per step and is LDWEIGHTS-bound (~650us).
Convergence is geometric (~0.55x/sweep, tanh saturation); 14 sweeps -> ~9e-3
end-to-end rel err (validated vs the exact reference on the real weights).

Sharding: expert-parallel, 2 encoders per core over 8 cores.  The decoder is
row(k)-sharded: each core computes full-width partial preactivations from its
local shard, combined with ReduceScatter(add) at the three layer boundaries;
the output layer's partials are written back per-core and the HOST sums the
8 partials.  Biases enter matmul-side via K=1 row-matmuls against a ones
vector (b/8 per core, summed by the collectives).  All weights/activations
fp16 on the PE (bf16 speed, 8x lower quantization noise), fp32 PSUM.

Schedule highlights:
- wff1/wff2 prefetched during the Jacobi sweeps (ff pool co-resident with the
  rnn pool; was: ff DMA stalled the FF phase ~35us).
- j-granular DVE-add + tanh in the sweep loop (act_j overlaps the next
  j-group's matmuls; cuts the per-sweep critical path ~0.8us).
- decoder weights staged late: wd0 into the freed rnn space during FF,
  wdm/wdo after the ff pool closes (hidden behind ReduceScatter 0).
- collective payloads fp16 (half the wire bytes of f32).
- single rotating PSUM pool for FF/decoder accumulation groups.

See kernel_v2.py docstring for the algorithm description.
"""

import sys
import numpy as np

sys.path.insert(0, "/opt/trn_rl_repo")

import ml_dtypes

E = 16
L = 3
D_IN = 32
D = 512
H_FF = 2048
D_ENC = 512
N_DEC = 4
H_DEC = 2048
D_OUT = 1024
T_FULL = 128
N_CORES = 8

E_LOC = E // N_CORES
DT = D // 128
NFT = H_FF // 128
NCAT = (L * D) // 128
NHD = H_DEC // 128
KD0 = (E_LOC * D_ENC) // 128
HD_SH = H_DEC // N_CORES
KDM = HD_SH // 128
DO_SH = D_OUT // N_CORES

S_SWEEPS = 14

F16 = ml_dtypes.float16 if hasattr(ml_dtypes, "float16") else np.float16


def _tile_kxm(w):
    """[K, M] -> [128, nk*nm*128] with col ((i*nm)+j)*128 : lhsT tile (i,j)."""
    K, M = w.shape
    nk, nm = K // 128, M // 128
    return np.ascontiguousarray(
        w.reshape(nk, 128, nm, 128).transpose(1, 0, 2, 3).reshape(128, nk * nm * 128)
    )


def _bias_cols(b):
    """[M] -> [128, M//128] with col j holding b[j*128:(j+1)*128]."""
    return np.ascontiguousarray(b.reshape(-1, 128).T)


def build_nc(t_steps):
    from concourse import bacc, mybir, tile

    F32 = mybir.dt.float32
    FP16 = mybir.dt.float16
    AF = mybir.ActivationFunctionType
    ADD = mybir.AluOpType.add
    T = t_steps

    nc = bacc.Bacc(None, num_devices=N_CORES)

    # ---- I/O ----------------------------------------------------------------
    xT = nc.dram_tensor("xT", [D_IN, T], F32, kind="ExternalInput")
    win0 = [nc.dram_tensor(f"win0_{k}", [D_IN, D], F32, kind="ExternalInput")
            for k in range(E_LOC)]
    wh = [nc.dram_tensor(f"wh_{k}", [128, L * DT * DT * 128], FP16, kind="ExternalInput")
          for k in range(E_LOC)]
    win = [nc.dram_tensor(f"win_{k}", [128, (L - 1) * DT * DT * 128], FP16,
                          kind="ExternalInput") for k in range(E_LOC)]
    b_rnn = [nc.dram_tensor(f"b_{k}", [128, L * DT], F32, kind="ExternalInput")
             for k in range(E_LOC)]
    wff1 = [nc.dram_tensor(f"wff1_{k}", [128, NCAT * NFT * 128], FP16, kind="ExternalInput")
            for k in range(E_LOC)]
    bff1 = [nc.dram_tensor(f"bff1_{k}", [128, NFT], F32, kind="ExternalInput")
            for k in range(E_LOC)]
    wff2 = [nc.dram_tensor(f"wff2_{k}", [128, NFT * DT * 128], FP16, kind="ExternalInput")
            for k in range(E_LOC)]
    bff2 = [nc.dram_tensor(f"bff2_{k}", [128, DT], F32, kind="ExternalInput")
            for k in range(E_LOC)]
    wd0 = nc.dram_tensor("wd0", [128, KD0 * NHD * 128], FP16, kind="ExternalInput")
    bd0r = nc.dram_tensor("bd0r", [1, NHD * 128], FP16, kind="ExternalInput")
    wdm = [nc.dram_tensor(f"wdm{m}", [128, KDM * NHD * 128], FP16, kind="ExternalInput")
           for m in range(N_DEC - 2)]
    bdmr = [nc.dram_tensor(f"bdmr{m}", [1, NHD * 128], FP16, kind="ExternalInput")
            for m in range(N_DEC - 2)]
    NDO = D_OUT // 128            # 8 output j-tiles
    wdo = nc.dram_tensor("wdo", [128, KDM * NDO * 128], FP16, kind="ExternalInput")
    bdor = nc.dram_tensor("bdor", [1, NDO * 128], FP16, kind="ExternalInput")
    # per-core output-layer PARTIAL; the host sums the 8 cores' tensors
    y_out = nc.dram_tensor("y_out", [D_OUT, T], F32, kind="ExternalOutput")

    cc_in = [nc.dram_tensor(f"cc_in{m}", [H_DEC, T], FP16) for m in range(N_DEC - 1)]
    rs_out = [nc.dram_tensor(f"rs_out{m}", [HD_SH, T], FP16) for m in range(N_DEC - 1)]

    RG = [list(range(N_CORES))]

    def colw(i, j, nm):
        return (i * nm + j) * 128

    LCH = DT * DT * 128

    with tile.TileContext(nc, num_cores=N_CORES) as tc:
        with (
            tc.tile_pool(name="persist", bufs=1) as persist,
            tc.tile_pool(name="ps_main", bufs=4, space="PSUM") as ps_main,
            tc.tile_pool(name="tmp", bufs=4) as tmp_pool,
        ):
            # --- small persistent tensors
            xT_sb = persist.tile([D_IN, T], F32, name="xT", tag="xT")
            nc.sync.dma_start(xT_sb[:], xT[:])
            win0_sb, b_sb, bff1_sb, bff2_sb = [], [], [], []
            u_sb, ench_sb = [], []
            hh = [[None] * L for _ in range(E_LOC)]
            for k in range(E_LOC):
                w0 = persist.tile([D_IN, D], F32, name=f"win0_{k}", tag=f"win0_{k}")
                nc.sync.dma_start(w0[:], win0[k][:])
                win0_sb.append(w0)
                bb = persist.tile([128, L * DT], F32, name=f"b_{k}", tag=f"b_{k}")
                nc.sync.dma_start(bb[:], b_rnn[k][:])
                b_sb.append(bb)
                b1 = persist.tile([128, NFT], F32, name=f"bff1_{k}", tag=f"bff1_{k}")
                nc.sync.dma_start(b1[:], bff1[k][:])
                bff1_sb.append(b1)
                b2 = persist.tile([128, DT], F32, name=f"bff2_{k}", tag=f"bff2_{k}")
                nc.sync.dma_start(b2[:], bff2[k][:])
                bff2_sb.append(b2)
                u_sb.append(persist.tile([128, DT, T], FP16, name=f"u_{k}", tag=f"u_{k}"))
                ench_sb.append(persist.tile([128, DT, T], FP16, name=f"enc_{k}", tag=f"enc_{k}"))
                for l in range(L):
                    hh[k][l] = [
                        persist.tile([128, DT, 1 + T], FP16, name=f"hh_{k}_{l}_{b}",
                                     tag=f"hh_{k}_{l}_{b}")
                        for b in range(2)
                    ]
                    nc.vector.memset(hh[k][l][0][:], 0.0)
                    nc.vector.memset(hh[k][l][1][:, :, 0:1], 0.0)
            ones_sb = persist.tile([1, T], FP16, name="ones", tag="ones")
            nc.vector.memset(ones_sb[:], 1.0)
            bd0r_sb = persist.tile([1, NHD * 128], FP16, name="bd0r", tag="bd0r")
            nc.sync.dma_start(bd0r_sb[:], bd0r[:])

            with tc.tile_pool(name="ff", bufs=1) as ff:
                wff1_sb, wff2_sb, ffs_sb = [], [], []
                ffacc_sb = []
                for k in range(E_LOC):
                    wff1_sb.append(ff.tile([128, NCAT * NFT * 128], FP16,
                                           name=f"wff1_{k}", tag=f"wff1_{k}"))
                    wff2_sb.append(ff.tile([128, NFT * DT * 128], FP16,
                                           name=f"wff2_{k}", tag=f"wff2_{k}"))
                    ffacc_sb.append(ff.tile([128, NFT, T], FP16,
                                            name=f"ffacc_{k}", tag=f"ffacc_{k}"))

                with (
                    tc.tile_pool(name="rnn", bufs=1) as rnn,
                    tc.tile_pool(name="ps_sw", bufs=2, space="PSUM") as ps_sw,
                ):
                    wh_sb, win_sb = [], []
                    for k in range(E_LOC):
                        wh_sb.append(rnn.tile([128, L * LCH], FP16, name=f"wh_{k}", tag=f"wh_{k}"))
                        win_sb.append(rnn.tile([128, (L - 1) * LCH], FP16, name=f"win_{k}",
                                               tag=f"win_{k}"))
                    for l in range(L):
                        for k in range(E_LOC):
                            nc.sync.dma_start(wh_sb[k][:, l * LCH:(l + 1) * LCH],
                                              wh[k][:, l * LCH:(l + 1) * LCH])
                            if l < L - 1:
                                nc.sync.dma_start(win_sb[k][:, l * LCH:(l + 1) * LCH],
                                                  win[k][:, l * LCH:(l + 1) * LCH])
                    # ff weights stream in during the sweeps
                    for k in range(E_LOC):
                        half = NCAT * NFT * 128 // 2
                        nc.sync.dma_start(wff1_sb[k][:, 0:half], wff1[k][:, 0:half])
                        nc.sync.dma_start(wff1_sb[k][:, half:], wff1[k][:, half:])
                        nc.sync.dma_start(wff2_sb[k][:], wff2[k][:])

                    hfin = [[None] * E_LOC for _ in range(L)]
                    for l in range(L):
                        for k in range(E_LOC):
                            psu = ps_sw.tile([128, DT, T], F32, name=f"ps{k}", tag=f"ps{k}")
                            if l == 0:
                                for j in range(DT):
                                    nc.tensor.matmul(psu[:, j, :],
                                                     win0_sb[k][:, j * 128:(j + 1) * 128],
                                                     xT_sb[:], start=True, stop=True)
                            else:
                                hprev = hfin[l - 1][k]
                                for j in range(DT):
                                    for i in range(DT):
                                        nc.tensor.matmul(
                                            psu[:, j, :],
                                            win_sb[k][:, colw((l - 1) * DT + i, j, DT):
                                                      colw((l - 1) * DT + i, j, DT) + 128],
                                            hprev[:, i, 1:1 + T],
                                            start=(i == 0), stop=(i == DT - 1))
                            for j in range(DT):
                                nc.vector.tensor_scalar_add(
                                    u_sb[k][:, j, :], psu[:, j, :],
                                    b_sb[k][:, l * DT + j:l * DT + j + 1])

                        for s in range(S_SWEEPS):
                            for k in range(E_LOC):
                                src = hh[k][l][s % 2]
                                dst = hh[k][l][(s + 1) % 2]
                                if s == 0:
                                    # zero state: sweep 0 is just tanh(u)
                                    nc.scalar.activation(dst[:, :, 1:1 + T], u_sb[k][:],
                                                         AF.Tanh)
                                    continue
                                ps = ps_sw.tile([128, DT, T], F32, name=f"ps{k}", tag=f"ps{k}")
                                for jp in range(DT // 2):
                                    for j in (2 * jp, 2 * jp + 1):
                                        for i in range(DT):
                                            nc.tensor.matmul(
                                                ps[:, j, :],
                                                wh_sb[k][:, colw(l * DT + i, j, DT):
                                                         colw(l * DT + i, j, DT) + 128],
                                                src[:, i, 0:T],
                                                start=(i == 0), stop=(i == DT - 1))
                                    tt = tmp_pool.tile([128, 2, T], FP16, name=f"tt{k}",
                                                       tag=f"tt{k}")
                                    nc.vector.tensor_add(tt[:], ps[:, 2 * jp:2 * jp + 2, :],
                                                         u_sb[k][:, 2 * jp:2 * jp + 2, :])
                                    nc.scalar.activation(dst[:, 2 * jp:2 * jp + 2, 1:1 + T],
                                                         tt[:], AF.Tanh)
                        for k in range(E_LOC):
                            hfin[l][k] = hh[k][l][S_SWEEPS % 2]

                # --- FF head + decoder layer-0 partial (rnn space now free) --
                with tc.tile_pool(name="dec_w", bufs=1) as dec_w:
                    ffs_sb = ffacc_sb   # gelu output overwrites the staging buffer
                    pd_sb = dec_w.tile([128, NHD, T], FP16, name="pd_sb", tag="pd_sb")
                    wd0_sb = dec_w.tile([128, KD0 * NHD * 128], FP16, name="wd0", tag="wd0")
                    csz = KD0 * NHD * 128 // 4
                    for ch in range(4):
                        nc.sync.dma_start(wd0_sb[:, ch * csz:(ch + 1) * csz],
                                          wd0[:, ch * csz:(ch + 1) * csz])

                    for k in range(E_LOC):
                        for g in range(NFT // DT):          # 4 m-tiles per psum bank
                            pf = ps_main.tile([128, DT, T], F32, name="pm", tag="pm")
                            for mi in range(DT):
                                m = g * DT + mi
                                idx = 0
                                for l in range(L):
                                    for j in range(DT):
                                        nc.tensor.matmul(
                                            pf[:, mi, :],
                                            wff1_sb[k][:, colw(l * DT + j, m, NFT):
                                                       colw(l * DT + j, m, NFT) + 128],
                                            hfin[l][k][:, j, 1:1 + T],
                                            start=(idx == 0), stop=(idx == NCAT - 1))
                                        idx += 1
                                nc.scalar.activation(ffs_sb[k][:, m, :], pf[:, mi, :],
                                                     AF.Gelu_apprx_tanh,
                                                     bias=bff1_sb[k][:, m:m + 1])
                        pf2 = ps_main.tile([128, DT, T], F32, name="pm", tag="pm")
                        for j in range(DT):
                            for i in range(NFT):
                                nc.tensor.matmul(
                                    pf2[:, j, :],
                                    wff2_sb[k][:, colw(i, j, DT):colw(i, j, DT) + 128],
                                    ffs_sb[k][:, i, :],
                                    start=(i == 0), stop=(i == NFT - 1))
                            nc.vector.tensor_scalar_add(ench_sb[k][:, j, :], pf2[:, j, :],
                                                        bff2_sb[k][:, j:j + 1])

                    # decoder layer 0: k-sharded partial over this core's encoders
                    for g in range(NHD // DT):
                        pd = ps_main.tile([128, DT, T], F32, name="pm", tag="pm")
                        for ji in range(DT):
                            j2 = g * DT + ji
                            nc.tensor.matmul(pd[:, ji, :],
                                             bd0r_sb[:, j2 * 128:(j2 + 1) * 128],
                                             ones_sb[:], start=True, stop=False)
                            for i in range(KD0):
                                nc.tensor.matmul(
                                    pd[:, ji, :],
                                    wd0_sb[:, colw(i, j2, NHD):colw(i, j2, NHD) + 128],
                                    ench_sb[i // DT][:, i % DT, :],
                                    start=False, stop=(i == KD0 - 1))
                        nc.vector.tensor_copy(pd_sb[:, g * DT:(g + 1) * DT, :], pd[:])
                        nc.sync.dma_start(
                            cc_in[0][g * 512:(g + 1) * 512, :].rearrange(
                                "(i p) t -> p i t", p=128),
                            pd_sb[:, g * DT:(g + 1) * DT, :])
                    nc.gpsimd.collective_compute(
                        "ReduceScatter", ADD, replica_groups=RG,
                        ins=[cc_in[0][:]], outs=[rs_out[0][:]])

            # --- decoder mid/out (ff space now free) -------------------------
            with tc.tile_pool(name="dec2", bufs=1) as dec2:
                wdm_sb, bdmr_sb = [], []
                for m in range(N_DEC - 2):
                    t_ = dec2.tile([128, KDM * NHD * 128], FP16, name=f"wdm{m}", tag=f"wdm{m}")
                    nc.sync.dma_start(t_[:], wdm[m][:])
                    wdm_sb.append(t_)
                    t_ = dec2.tile([1, NHD * 128], FP16, name=f"bdmr{m}", tag=f"bdmr{m}")
                    nc.sync.dma_start(t_[:], bdmr[m][:])
                    bdmr_sb.append(t_)
                wdo_sb = dec2.tile([128, KDM * NDO * 128], FP16, name="wdo", tag="wdo")
                nc.sync.dma_start(wdo_sb[:], wdo[:])
                bdor_sb = dec2.tile([1, NDO * 128], FP16, name="bdor", tag="bdor")
                nc.sync.dma_start(bdor_sb[:], bdor[:])
                pd_sb = dec2.tile([128, NHD, T], FP16, name="pd_sb2", tag="pd_sb2")

                for m in range(N_DEC - 2):
                    zin = dec2.tile([128, KDM, T], FP16, name=f"zin{m}", tag=f"zin{m}")
                    zloc = dec2.tile([128, KDM, T], FP16, name=f"z{m}", tag=f"z{m}")
                    for i in range(KDM):
                        nc.sync.dma_start(
                            zin[:, i, :],
                            rs_out[m][i * 128:(i + 1) * 128, :].rearrange(
                                "(i p) t -> p i t", p=128))
                        nc.scalar.activation(zloc[:, i, :], zin[:, i, :], AF.Tanh)
                    pms = []
                    for g in range(NHD // DT):       # bias rows: no dep on the RS
                        pm = ps_main.tile([128, DT, T], F32, name="pm", tag="pm")
                        pms.append(pm)
                        for ji in range(DT):
                            j2 = g * DT + ji
                            nc.tensor.matmul(pm[:, ji, :],
                                             bdmr_sb[m][:, j2 * 128:(j2 + 1) * 128],
                                             ones_sb[:], start=True, stop=False)
                    for g in range(NHD // DT):
                        pm = pms[g]
                        for ji in range(DT):
                            j2 = g * DT + ji
                            for i in range(KDM):
                                nc.tensor.matmul(
                                    pm[:, ji, :],
                                    wdm_sb[m][:, colw(i, j2, NHD):colw(i, j2, NHD) + 128],
                                    zloc[:, i, :],
                                    start=False, stop=(i == KDM - 1))
                        nc.vector.tensor_copy(pd_sb[:, g * DT:(g + 1) * DT, :], pm[:])
                        nc.sync.dma_start(
                            cc_in[m + 1][g * 512:(g + 1) * 512, :].rearrange(
                                "(i p) t -> p i t", p=128),
                            pd_sb[:, g * DT:(g + 1) * DT, :])
                    nc.gpsimd.collective_compute(
                        "ReduceScatter", ADD, replica_groups=RG,
                        ins=[cc_in[m + 1][:]], outs=[rs_out[m + 1][:]])

                # output layer: k-sharded partial, ReduceScatter straight into y_out
                zin3 = dec2.tile([128, KDM, T], FP16, name="zin3", tag="zin3")
                z3 = dec2.tile([128, KDM, T], FP16, name="z3", tag="z3")
                for i in range(KDM):
                    nc.sync.dma_start(
                        zin3[:, i, :],
                        rs_out[N_DEC - 2][i * 128:(i + 1) * 128, :].rearrange(
                            "(i p) t -> p i t", p=128))
                    nc.scalar.activation(z3[:, i, :], zin3[:, i, :], AF.Tanh)
                yp_sb = dec2.tile([128, NDO, T], F32, name="yp_sb", tag="yp_sb")
                pys = []
                for g in range(NDO // DT):
                    py = ps_main.tile([128, DT, T], F32, name="pm", tag="pm")
                    pys.append(py)
                    for ji in range(DT):
                        j2 = g * DT + ji
                        nc.tensor.matmul(py[:, ji, :],
                                         bdor_sb[:, j2 * 128:(j2 + 1) * 128],
                                         ones_sb[:], start=True, stop=False)
                for g in range(NDO // DT):
                    py = pys[g]
                    for ji in range(DT):
                        j2 = g * DT + ji
                        for i in range(KDM):
                            nc.tensor.matmul(
                                py[:, ji, :],
                                wdo_sb[:, colw(i, j2, NDO):colw(i, j2, NDO) + 128],
                                z3[:, i, :],
                                start=False, stop=(i == KDM - 1))
                    nc.vector.tensor_copy(yp_sb[:, g * DT:(g + 1) * DT, :], py[:])
                    nc.sync.dma_start(
                        y_out[g * 512:(g + 1) * 512, :].rearrange("(i p) t -> p i t", p=128),
                        yp_sb[:, g * DT:(g + 1) * DT, :])

    nc.compile()
    return nc


def prep_inputs(inputs, t_steps):
    """Build the 8 per-core input maps from full numpy inputs."""
    T = t_steps
    f32 = lambda a: np.asarray(a, np.float32)
    x = f32(inputs["x"])
    W_in0, Wh0, b0 = f32(inputs["W_in0"]), f32(inputs["Wh0"]), f32(inputs["b0"])
    W_in_rest, Wh_rest, b_rest = (f32(inputs["W_in_rest"]), f32(inputs["Wh_rest"]),
                                  f32(inputs["b_rest"]))
    W_ff1, b_ff1 = f32(inputs["W_ff1"]), f32(inputs["b_ff1"])
    W_ff2, b_ff2 = f32(inputs["W_ff2"]), f32(inputs["b_ff2"])
    W_d0, b_d0 = f32(inputs["W_d0"]), f32(inputs["b_d0"])
    W_dmid, b_dmid = f32(inputs["W_dmid"]), f32(inputs["b_dmid"])
    W_dout, b_dout = f32(inputs["W_dout"]), f32(inputs["b_dout"])

    xT = np.ascontiguousarray(x[0, :T].T)
    in_maps = []
    for c in range(N_CORES):
        m = {"xT": xT}
        for k in range(E_LOC):
            e = E_LOC * c + k
            m[f"win0_{k}"] = np.ascontiguousarray(W_in0[e])
            wh_all = np.concatenate([Wh0[e][None], Wh_rest[e]], 0)
            m[f"wh_{k}"] = _tile_kxm(wh_all.reshape(L * D, D)).astype(F16)
            m[f"win_{k}"] = _tile_kxm(W_in_rest[e].reshape((L - 1) * D, D)).astype(F16)
            b_all = np.concatenate([b0[e][None], b_rest[e]], 0).reshape(-1)
            m[f"b_{k}"] = _bias_cols(b_all)
            m[f"wff1_{k}"] = _tile_kxm(W_ff1[e]).astype(F16)
            m[f"bff1_{k}"] = _bias_cols(b_ff1[e])
            m[f"wff2_{k}"] = _tile_kxm(W_ff2[e]).astype(F16)
            m[f"bff2_{k}"] = _bias_cols(b_ff2[e])
        m["wd0"] = _tile_kxm(W_d0[c * E_LOC * D_ENC:(c + 1) * E_LOC * D_ENC, :]).astype(F16)
        m["bd0r"] = np.ascontiguousarray((b_d0 / N_CORES)[None, :]).astype(F16)
        for mm in range(N_DEC - 2):
            m[f"wdm{mm}"] = _tile_kxm(W_dmid[mm][c * HD_SH:(c + 1) * HD_SH, :]).astype(F16)
            m[f"bdmr{mm}"] = np.ascontiguousarray((b_dmid[mm] / N_CORES)[None, :]).astype(F16)
        m["wdo"] = _tile_kxm(W_dout[c * HD_SH:(c + 1) * HD_SH, :]).astype(F16)
        m["bdor"] = np.ascontiguousarray((b_dout / N_CORES)[None, :]).astype(F16)
        in_maps.append(m)
    return in_maps


def run(inputs, t_steps=T_FULL, trace=False):
    from concourse.bass_utils import run_bass_kernel_spmd

    nc = build_nc(t_steps)
    in_maps = prep_inputs(inputs, t_steps)
    res = run_bass_kernel_spmd(nc, in_maps, list(range(N_CORES)), trace=trace)
    acc = np.zeros((D_OUT, t_steps), np.float32)
    for c in range(N_CORES):
        acc += np.asarray(res.results[c]["y_out"], np.float32)
    return acc.T[None], res


def kernel(**inputs):
    y, _ = run(inputs, T_FULL, trace=False)
    return y


# revision 4
# speedup vs baseline: 1.0246x; 1.0126x over previous
"""HRNN Trainium2 kernel: Jacobi trajectory iteration for the recurrence.

Algorithm: the tanh-RNN recurrence h_t = tanh(Wh h_{t-1} + u_t) is solved per
layer by full-trajectory Jacobi fixed-point sweeps H^(m+1) = tanh(Wh H^m + U)
(14 sweeps) instead of T=128 sequential steps.  Each sweep is a batched matmul
over all T timesteps, amortizing PE weight loads 128-way; the sequential
formulation reloads 32 weight tiles per step and is LDWEIGHTS-bound (~650us).
Convergence is geometric (~0.55x/sweep, tanh saturation); 14 sweeps -> ~9e-3
end-to-end rel err (validated vs the exact reference on the real weights).
Sweep 0 from the zero state is act-only: tanh(U).

Sharding: expert-parallel, 2 encoders per core over 8 cores.  The decoder is
row(k)-sharded: each core computes full-width partial preactivations from its
local shard, combined with ReduceScatter(add) at the three layer boundaries;
the output layer's partials are written back per-core and the HOST sums the
8 partials (no final collective).  Biases enter matmul-side via K=1
row-matmuls against a ones vector (b/8 per core, summed by the collectives).
All weights/activations fp16 on the PE (bf16 speed, 8x lower quantization
noise), fp32 PSUM accumulation.

Schedule highlights:
- wff1/wff2 prefetched during the Jacobi sweeps (ff pool co-resident with the
  rnn pool; was: ff DMA stalled the FF phase ~35us).
- j-granular DVE-add + tanh in the sweep loop (act_j overlaps the next
  j-group's matmuls; cuts the per-sweep critical path ~0.8us).
- decoder weights staged late: wd0 into the freed rnn space during FF,
  wdm/wdo after the ff pool closes (hidden behind ReduceScatter 0).
- collective payloads fp16 (half the wire bytes of f32).
- single rotating PSUM pool for FF/decoder accumulation groups.

See kernel_v2.py docstring for the algorithm description.
"""

import sys
import numpy as np

sys.path.insert(0, "/opt/trn_rl_repo")

import ml_dtypes

E = 16
L = 3
D_IN = 32
D = 512
H_FF = 2048
D_ENC = 512
N_DEC = 4
H_DEC = 2048
D_OUT = 1024
T_FULL = 128
N_CORES = 8

E_LOC = E // N_CORES
DT = D // 128
NFT = H_FF // 128
NCAT = (L * D) // 128
NHD = H_DEC // 128
KD0 = (E_LOC * D_ENC) // 128
HD_SH = H_DEC // N_CORES
KDM = HD_SH // 128
DO_SH = D_OUT // N_CORES

S_SWEEPS = 14

F16 = ml_dtypes.float16 if hasattr(ml_dtypes, "float16") else np.float16


def _tile_kxm(w):
    """[K, M] -> [128, nk*nm*128] with col ((i*nm)+j)*128 : lhsT tile (i,j)."""
    K, M = w.shape
    nk, nm = K // 128, M // 128
    return np.ascontiguousarray(
        w.reshape(nk, 128, nm, 128).transpose(1, 0, 2, 3).reshape(128, nk * nm * 128)
    )


def _bias_cols(b):
    """[M] -> [128, M//128] with col j holding b[j*128:(j+1)*128]."""
    return np.ascontiguousarray(b.reshape(-1, 128).T)


def build_nc(t_steps):
    from concourse import bacc, mybir, tile

    F32 = mybir.dt.float32
    FP16 = mybir.dt.float16
    AF = mybir.ActivationFunctionType
    ADD = mybir.AluOpType.add
    T = t_steps

    nc = bacc.Bacc(None, num_devices=N_CORES)

    # ---- I/O ----------------------------------------------------------------
    xT = nc.dram_tensor("xT", [D_IN, T], F32, kind="ExternalInput")
    win0 = [nc.dram_tensor(f"win0_{k}", [D_IN, D], F32, kind="ExternalInput")
            for k in range(E_LOC)]
    wh = [nc.dram_tensor(f"wh_{k}", [128, L * DT * DT * 128], FP16, kind="ExternalInput")
          for k in range(E_LOC)]
    win = [nc.dram_tensor(f"win_{k}", [128, (L - 1) * DT * DT * 128], FP16,
                          kind="ExternalInput") for k in range(E_LOC)]
    b_rnn = [nc.dram_tensor(f"b_{k}", [128, L * DT], F32, kind="ExternalInput")
             for k in range(E_LOC)]
    wff1 = [nc.dram_tensor(f"wff1_{k}", [128, NCAT * NFT * 128], FP16, kind="ExternalInput")
            for k in range(E_LOC)]
    bff1 = [nc.dram_tensor(f"bff1_{k}", [128, NFT], F32, kind="ExternalInput")
            for k in range(E_LOC)]
    wff2 = [nc.dram_tensor(f"wff2_{k}", [128, NFT * DT * 128], FP16, kind="ExternalInput")
            for k in range(E_LOC)]
    bff2 = [nc.dram_tensor(f"bff2_{k}", [128, DT], F32, kind="ExternalInput")
            for k in range(E_LOC)]
    wd0 = nc.dram_tensor("wd0", [128, KD0 * NHD * 128], FP16, kind="ExternalInput")
    bd0r = nc.dram_tensor("bd0r", [1, NHD * 128], FP16, kind="ExternalInput")
    wdm = [nc.dram_tensor(f"wdm{m}", [128, KDM * NHD * 128], FP16, kind="ExternalInput")
           for m in range(N_DEC - 2)]
    bdmr = [nc.dram_tensor(f"bdmr{m}", [1, NHD * 128], FP16, kind="ExternalInput")
            for m in range(N_DEC - 2)]
    NDO = D_OUT // 128            # 8 output j-tiles
    wdo = nc.dram_tensor("wdo", [128, KDM * NDO * 128], FP16, kind="ExternalInput")
    bdor = nc.dram_tensor("bdor", [1, NDO * 128], FP16, kind="ExternalInput")
    # per-core output-layer PARTIAL; the host sums the 8 cores' tensors
    y_out = nc.dram_tensor("y_out", [D_OUT, T], F32, kind="ExternalOutput")

    cc_in = [nc.dram_tensor(f"cc_in{m}", [H_DEC, T], FP16) for m in range(N_DEC - 1)]
    rs_out = [nc.dram_tensor(f"rs_out{m}", [HD_SH, T], FP16) for m in range(N_DEC - 1)]

    RG = [list(range(N_CORES))]

    def colw(i, j, nm):
        return (i * nm + j) * 128

    LCH = DT * DT * 128

    with tile.TileContext(nc, num_cores=N_CORES) as tc:
        with (
            tc.tile_pool(name="persist", bufs=1) as persist,
            tc.tile_pool(name="ps_main", bufs=4, space="PSUM") as ps_main,
            tc.tile_pool(name="tmp", bufs=4) as tmp_pool,
        ):
            # --- small persistent tensors
            xT_sb = persist.tile([D_IN, T], F32, name="xT", tag="xT")
            nc.sync.dma_start(xT_sb[:], xT[:])
            win0_sb, b_sb, bff1_sb, bff2_sb = [], [], [], []
            u_sb, ench_sb = [], []
            hh = [[None] * L for _ in range(E_LOC)]
            for k in range(E_LOC):
                w0 = persist.tile([D_IN, D], F32, name=f"win0_{k}", tag=f"win0_{k}")
                nc.sync.dma_start(w0[:], win0[k][:])
                win0_sb.append(w0)
                bb = persist.tile([128, L * DT], F32, name=f"b_{k}", tag=f"b_{k}")
                nc.sync.dma_start(bb[:], b_rnn[k][:])
                b_sb.append(bb)
                b1 = persist.tile([128, NFT], F32, name=f"bff1_{k}", tag=f"bff1_{k}")
                nc.sync.dma_start(b1[:], bff1[k][:])
                bff1_sb.append(b1)
                b2 = persist.tile([128, DT], F32, name=f"bff2_{k}", tag=f"bff2_{k}")
                nc.sync.dma_start(b2[:], bff2[k][:])
                bff2_sb.append(b2)
                u_sb.append(persist.tile([128, DT, T], FP16, name=f"u_{k}", tag=f"u_{k}"))
                ench_sb.append(persist.tile([128, DT, T], FP16, name=f"enc_{k}", tag=f"enc_{k}"))
                for l in range(L):
                    hh[k][l] = [
                        persist.tile([128, DT, 1 + T], FP16, name=f"hh_{k}_{l}_{b}",
                                     tag=f"hh_{k}_{l}_{b}")
                        for b in range(2)
                    ]
                    nc.vector.memset(hh[k][l][0][:], 0.0)
                    nc.vector.memset(hh[k][l][1][:, :, 0:1], 0.0)
            ones_sb = persist.tile([1, T], FP16, name="ones", tag="ones")
            nc.vector.memset(ones_sb[:], 1.0)
            bd0r_sb = persist.tile([1, NHD * 128], FP16, name="bd0r", tag="bd0r")
            nc.sync.dma_start(bd0r_sb[:], bd0r[:])

            with tc.tile_pool(name="ff", bufs=1) as ff:
                wff1_sb, wff2_sb, ffs_sb = [], [], []
                ffacc_sb = []
                for k in range(E_LOC):
                    wff1_sb.append(ff.tile([128, NCAT * NFT * 128], FP16,
                                           name=f"wff1_{k}", tag=f"wff1_{k}"))
                    wff2_sb.append(ff.tile([128, NFT * DT * 128], FP16,
                                           name=f"wff2_{k}", tag=f"wff2_{k}"))
                    ffacc_sb.append(ff.tile([128, NFT, T], FP16,
                                            name=f"ffacc_{k}", tag=f"ffacc_{k}"))

                with (
                    tc.tile_pool(name="rnn", bufs=1) as rnn,
                    tc.tile_pool(name="ps_sw", bufs=2, space="PSUM") as ps_sw,
                ):
                    wh_sb, win_sb = [], []
                    for k in range(E_LOC):
                        wh_sb.append(rnn.tile([128, L * LCH], FP16, name=f"wh_{k}", tag=f"wh_{k}"))
                        win_sb.append(rnn.tile([128, (L - 1) * LCH], FP16, name=f"win_{k}",
                                               tag=f"win_{k}"))
                    for l in range(L):
                        for k in range(E_LOC):
                            nc.sync.dma_start(wh_sb[k][:, l * LCH:(l + 1) * LCH],
                                              wh[k][:, l * LCH:(l + 1) * LCH])
                            if l < L - 1:
                                nc.sync.dma_start(win_sb[k][:, l * LCH:(l + 1) * LCH],
                                                  win[k][:, l * LCH:(l + 1) * LCH])
                    # ff weights stream in during the sweeps
                    for k in range(E_LOC):
                        half = NCAT * NFT * 128 // 2
                        nc.sync.dma_start(wff1_sb[k][:, 0:half], wff1[k][:, 0:half])
                        nc.sync.dma_start(wff1_sb[k][:, half:], wff1[k][:, half:])
                        nc.sync.dma_start(wff2_sb[k][:], wff2[k][:])

                    hfin = [[None] * E_LOC for _ in range(L)]
                    for l in range(L):
                        for k in range(E_LOC):
                            psu = ps_sw.tile([128, DT, T], F32, name=f"ps{k}", tag=f"ps{k}")
                            if l == 0:
                                for j in range(DT):
                                    nc.tensor.matmul(psu[:, j, :],
                                                     win0_sb[k][:, j * 128:(j + 1) * 128],
                                                     xT_sb[:], start=True, stop=True)
                            else:
                                hprev = hfin[l - 1][k]
                                for j in range(DT):
                                    for i in range(DT):
                                        nc.tensor.matmul(
                                            psu[:, j, :],
                                            win_sb[k][:, colw((l - 1) * DT + i, j, DT):
                                                      colw((l - 1) * DT + i, j, DT) + 128],
                                            hprev[:, i, 1:1 + T],
                                            start=(i == 0), stop=(i == DT - 1))
                            for j in range(DT):
                                nc.vector.tensor_scalar_add(
                                    u_sb[k][:, j, :], psu[:, j, :],
                                    b_sb[k][:, l * DT + j:l * DT + j + 1])

                        for s in range(S_SWEEPS):
                            korder = range(E_LOC) if s % 2 == 0 else range(E_LOC - 1, -1, -1)
                            for k in korder:
                                src = hh[k][l][s % 2]
                                dst = hh[k][l][(s + 1) % 2]
                                if s == 0:
                                    # zero state: sweep 0 is just tanh(u)
                                    nc.scalar.activation(dst[:, :, 1:1 + T], u_sb[k][:],
                                                         AF.Tanh)
                                    continue
                                ps = ps_sw.tile([128, DT, T], F32, name=f"ps{k}", tag=f"ps{k}")
                                for jp in range(DT // 2):
                                    for j in (2 * jp, 2 * jp + 1):
                                        for i in range(DT):
                                            nc.tensor.matmul(
                                                ps[:, j, :],
                                                wh_sb[k][:, colw(l * DT + i, j, DT):
                                                         colw(l * DT + i, j, DT) + 128],
                                                src[:, i, 0:T],
                                                start=(i == 0), stop=(i == DT - 1))
                                    tt = tmp_pool.tile([128, 2, T], FP16, name=f"tt{k}",
                                                       tag=f"tt{k}")
                                    nc.vector.tensor_add(tt[:], ps[:, 2 * jp:2 * jp + 2, :],
                                                         u_sb[k][:, 2 * jp:2 * jp + 2, :])
                                    nc.scalar.activation(dst[:, 2 * jp:2 * jp + 2, 1:1 + T],
                                                         tt[:], AF.Tanh)
                        for k in range(E_LOC):
                            hfin[l][k] = hh[k][l][S_SWEEPS % 2]

                # --- FF head + decoder layer-0 partial (rnn space now free) --
                with tc.tile_pool(name="dec_w", bufs=1) as dec_w:
                    ffs_sb = ffacc_sb   # gelu output overwrites the staging buffer
                    pd_sb = dec_w.tile([128, NHD, T], FP16, name="pd_sb", tag="pd_sb")
                    wd0_sb = dec_w.tile([128, KD0 * NHD * 128], FP16, name="wd0", tag="wd0")
                    csz = KD0 * NHD * 128 // 4
                    for ch in range(4):
                        nc.sync.dma_start(wd0_sb[:, ch * csz:(ch + 1) * csz],
                                          wd0[:, ch * csz:(ch + 1) * csz])

                    for k in range(E_LOC):
                        for g in range(NFT // DT):          # 4 m-tiles per psum bank
                            pf = ps_main.tile([128, DT, T], F32, name="pm", tag="pm")
                            for mi in range(DT):
                                m = g * DT + mi
                                idx = 0
                                for l in range(L):
                                    for j in range(DT):
                                        nc.tensor.matmul(
                                            pf[:, mi, :],
                                            wff1_sb[k][:, colw(l * DT + j, m, NFT):
                                                       colw(l * DT + j, m, NFT) + 128],
                                            hfin[l][k][:, j, 1:1 + T],
                                            start=(idx == 0), stop=(idx == NCAT - 1))
                                        idx += 1
                                nc.scalar.activation(ffs_sb[k][:, m, :], pf[:, mi, :],
                                                     AF.Gelu_apprx_tanh,
                                                     bias=bff1_sb[k][:, m:m + 1])
                        pf2 = ps_main.tile([128, DT, T], F32, name="pm", tag="pm")
                        for j in range(DT):
                            for i in range(NFT):
                                nc.tensor.matmul(
                                    pf2[:, j, :],
                                    wff2_sb[k][:, colw(i, j, DT):colw(i, j, DT) + 128],
                                    ffs_sb[k][:, i, :],
                                    start=(i == 0), stop=(i == NFT - 1))
                            nc.vector.tensor_scalar_add(ench_sb[k][:, j, :], pf2[:, j, :],
                                                        bff2_sb[k][:, j:j + 1])

                    # decoder layer 0: k-sharded partial over this core's encoders
                    for g in range(NHD // DT):
                        pd = ps_main.tile([128, DT, T], F32, name="pm", tag="pm")
                        for ji in range(DT):
                            j2 = g * DT + ji
                            nc.tensor.matmul(pd[:, ji, :],
                                             bd0r_sb[:, j2 * 128:(j2 + 1) * 128],
                                             ones_sb[:], start=True, stop=False)
                            for i in range(KD0):
                                nc.tensor.matmul(
                                    pd[:, ji, :],
                                    wd0_sb[:, colw(i, j2, NHD):colw(i, j2, NHD) + 128],
                                    ench_sb[i // DT][:, i % DT, :],
                                    start=False, stop=(i == KD0 - 1))
                        nc.vector.tensor_copy(pd_sb[:, g * DT:(g + 1) * DT, :], pd[:])
                        nc.sync.dma_start(
                            cc_in[0][g * 512:(g + 1) * 512, :].rearrange(
                                "(i p) t -> p i t", p=128),
                            pd_sb[:, g * DT:(g + 1) * DT, :])
                    nc.gpsimd.collective_compute(
                        "ReduceScatter", ADD, replica_groups=RG,
                        ins=[cc_in[0][:]], outs=[rs_out[0][:]])

            # --- decoder mid/out (ff space now free) -------------------------
            with tc.tile_pool(name="dec2", bufs=1) as dec2:
                wdm_sb, bdmr_sb = [], []
                for m in range(N_DEC - 2):
                    t_ = dec2.tile([128, KDM * NHD * 128], FP16, name=f"wdm{m}", tag=f"wdm{m}")
                    nc.sync.dma_start(t_[:], wdm[m][:])
                    wdm_sb.append(t_)
                    t_ = dec2.tile([1, NHD * 128], FP16, name=f"bdmr{m}", tag=f"bdmr{m}")
                    nc.sync.dma_start(t_[:], bdmr[m][:])
                    bdmr_sb.append(t_)
                wdo_sb = dec2.tile([128, KDM * NDO * 128], FP16, name="wdo", tag="wdo")
                nc.sync.dma_start(wdo_sb[:], wdo[:])
                bdor_sb = dec2.tile([1, NDO * 128], FP16, name="bdor", tag="bdor")
                nc.sync.dma_start(bdor_sb[:], bdor[:])
                pd_sb = dec2.tile([128, NHD, T], FP16, name="pd_sb2", tag="pd_sb2")

                for m in range(N_DEC - 2):
                    zin = dec2.tile([128, KDM, T], FP16, name=f"zin{m}", tag=f"zin{m}")
                    zloc = dec2.tile([128, KDM, T], FP16, name=f"z{m}", tag=f"z{m}")
                    for i in range(KDM):
                        nc.sync.dma_start(
                            zin[:, i, :],
                            rs_out[m][i * 128:(i + 1) * 128, :].rearrange(
                                "(i p) t -> p i t", p=128))
                        nc.scalar.activation(zloc[:, i, :], zin[:, i, :], AF.Tanh)
                    pms = []
                    for g in range(NHD // DT):       # bias rows: no dep on the RS
                        pm = ps_main.tile([128, DT, T], F32, name="pm", tag="pm")
                        pms.append(pm)
                        for ji in range(DT):
                            j2 = g * DT + ji
                            nc.tensor.matmul(pm[:, ji, :],
                                             bdmr_sb[m][:, j2 * 128:(j2 + 1) * 128],
                                             ones_sb[:], start=True, stop=False)
                    for g in range(NHD // DT):
                        pm = pms[g]
                        for ji in range(DT):
                            j2 = g * DT + ji
                            for i in range(KDM):
                                nc.tensor.matmul(
                                    pm[:, ji, :],
                                    wdm_sb[m][:, colw(i, j2, NHD):colw(i, j2, NHD) + 128],
                                    zloc[:, i, :],
                                    start=False, stop=(i == KDM - 1))
                        nc.vector.tensor_copy(pd_sb[:, g * DT:(g + 1) * DT, :], pm[:])
                        nc.sync.dma_start(
                            cc_in[m + 1][g * 512:(g + 1) * 512, :].rearrange(
                                "(i p) t -> p i t", p=128),
                            pd_sb[:, g * DT:(g + 1) * DT, :])
                    nc.gpsimd.collective_compute(
                        "ReduceScatter", ADD, replica_groups=RG,
                        ins=[cc_in[m + 1][:]], outs=[rs_out[m + 1][:]])

                # output layer: k-sharded partial, ReduceScatter straight into y_out
                zin3 = dec2.tile([128, KDM, T], FP16, name="zin3", tag="zin3")
                z3 = dec2.tile([128, KDM, T], FP16, name="z3", tag="z3")
                for i in range(KDM):
                    nc.sync.dma_start(
                        zin3[:, i, :],
                        rs_out[N_DEC - 2][i * 128:(i + 1) * 128, :].rearrange(
                            "(i p) t -> p i t", p=128))
                    nc.scalar.activation(z3[:, i, :], zin3[:, i, :], AF.Tanh)
                yp_sb = dec2.tile([128, NDO, T], F32, name="yp_sb", tag="yp_sb")
                pys = []
                for g in range(NDO // DT):
                    py = ps_main.tile([128, DT, T], F32, name="pm", tag="pm")
                    pys.append(py)
                    for ji in range(DT):
                        j2 = g * DT + ji
                        nc.tensor.matmul(py[:, ji, :],
                                         bdor_sb[:, j2 * 128:(j2 + 1) * 128],
                                         ones_sb[:], start=True, stop=False)
                for g in range(NDO // DT):
                    py = pys[g]
                    for ji in range(DT):
                        j2 = g * DT + ji
                        for i in range(KDM):
                            nc.tensor.matmul(
                                py[:, ji, :],
                                wdo_sb[:, colw(i, j2, NDO):colw(i, j2, NDO) + 128],
                                z3[:, i, :],
                                start=False, stop=(i == KDM - 1))
                    nc.vector.tensor_copy(yp_sb[:, g * DT:(g + 1) * DT, :], py[:])
                    nc.sync.dma_start(
                        y_out[g * 512:(g + 1) * 512, :].rearrange("(i p) t -> p i t", p=128),
                        yp_sb[:, g * DT:(g + 1) * DT, :])

    nc.compile()
    return nc


def prep_inputs(inputs, t_steps):
    """Build the 8 per-core input maps from full numpy inputs."""
    T = t_steps
    f32 = lambda a: np.asarray(a, np.float32)
    x = f32(inputs["x"])
    W_in0, Wh0, b0 = f32(inputs["W_in0"]), f32(inputs["Wh0"]), f32(inputs["b0"])
    W_in_rest, Wh_rest, b_rest = (f32(inputs["W_in_rest"]), f32(inputs["Wh_rest"]),
                                  f32(inputs["b_rest"]))
    W_ff1, b_ff1 = f32(inputs["W_ff1"]), f32(inputs["b_ff1"])
    W_ff2, b_ff2 = f32(inputs["W_ff2"]), f32(inputs["b_ff2"])
    W_d0, b_d0 = f32(inputs["W_d0"]), f32(inputs["b_d0"])
    W_dmid, b_dmid = f32(inputs["W_dmid"]), f32(inputs["b_dmid"])
    W_dout, b_dout = f32(inputs["W_dout"]), f32(inputs["b_dout"])

    xT = np.ascontiguousarray(x[0, :T].T)
    in_maps = []
    for c in range(N_CORES):
        m = {"xT": xT}
        for k in range(E_LOC):
            e = E_LOC * c + k
            m[f"win0_{k}"] = np.ascontiguousarray(W_in0[e])
            wh_all = np.concatenate([Wh0[e][None], Wh_rest[e]], 0)
            m[f"wh_{k}"] = _tile_kxm(wh_all.reshape(L * D, D)).astype(F16)
            m[f"win_{k}"] = _tile_kxm(W_in_rest[e].reshape((L - 1) * D, D)).astype(F16)
            b_all = np.concatenate([b0[e][None], b_rest[e]], 0).reshape(-1)
            m[f"b_{k}"] = _bias_cols(b_all)
            m[f"wff1_{k}"] = _tile_kxm(W_ff1[e]).astype(F16)
            m[f"bff1_{k}"] = _bias_cols(b_ff1[e])
            m[f"wff2_{k}"] = _tile_kxm(W_ff2[e]).astype(F16)
            m[f"bff2_{k}"] = _bias_cols(b_ff2[e])
        m["wd0"] = _tile_kxm(W_d0[c * E_LOC * D_ENC:(c + 1) * E_LOC * D_ENC, :]).astype(F16)
        m["bd0r"] = np.ascontiguousarray((b_d0 / N_CORES)[None, :]).astype(F16)
        for mm in range(N_DEC - 2):
            m[f"wdm{mm}"] = _tile_kxm(W_dmid[mm][c * HD_SH:(c + 1) * HD_SH, :]).astype(F16)
            m[f"bdmr{mm}"] = np.ascontiguousarray((b_dmid[mm] / N_CORES)[None, :]).astype(F16)
        m["wdo"] = _tile_kxm(W_dout[c * HD_SH:(c + 1) * HD_SH, :]).astype(F16)
        m["bdor"] = np.ascontiguousarray((b_dout / N_CORES)[None, :]).astype(F16)
        in_maps.append(m)
    return in_maps


def run(inputs, t_steps=T_FULL, trace=False):
    from concourse.bass_utils import run_bass_kernel_spmd

    nc = build_nc(t_steps)
    in_maps = prep_inputs(inputs, t_steps)
    res = run_bass_kernel_spmd(nc, in_maps, list(range(N_CORES)), trace=trace)
    acc = np.zeros((D_OUT, t_steps), np.float32)
    for c in range(N_CORES):
        acc += np.asarray(res.results[c]["y_out"], np.float32)
    return acc.T[None], res


def kernel(**inputs):
    y, _ = run(inputs, T_FULL, trace=False)
    return y


# revision 5
# speedup vs baseline: 1.0364x; 1.0115x over previous
"""HRNN Trainium2 kernel: Jacobi trajectory iteration for the recurrence.

Algorithm: the tanh-RNN recurrence h_t = tanh(Wh h_{t-1} + u_t) is solved per
layer by full-trajectory Jacobi fixed-point sweeps H^(m+1) = tanh(Wh H^m + U)
(14 sweeps) instead of T=128 sequential steps.  Each sweep is a batched matmul
over all T timesteps, amortizing PE weight loads 128-way; the sequential
formulation reloads 32 weight tiles per step and is LDWEIGHTS-bound (~650us).
Convergence is geometric (~0.55x/sweep, tanh saturation); 14 sweeps -> ~9e-3
end-to-end rel err (validated vs the exact reference on the real weights).
Sweep 0 from the zero state is act-only: tanh(U).

Per-sweep dataflow is asymmetric across the two local encoders to balance
engines: encoder 0 goes PSUM -> DVE add(+U) -> ACT tanh; encoder 1 injects U
into PSUM via an identity matmul and ACT reads PSUM directly (shorter chain,
no DVE hop; the sweeps alternate encoder emission order to de-phase engine
collisions).

Sharding: expert-parallel, 2 encoders per core over 8 cores.  The decoder is
row(k)-sharded: each core computes full-width partial preactivations from its
local shard, combined with ReduceScatter(add) at the three layer boundaries;
the output layer's partials are written back per-core and the HOST sums the
8 partials (no final collective).  Biases enter matmul-side via K=1
row-matmuls against a ones vector (b/8 per core, summed by the collectives).
All weights/activations fp16 on the PE (bf16 speed, 8x lower quantization
noise), fp32 PSUM accumulation.

Schedule highlights:
- wff1/wff2 prefetched during the Jacobi sweeps (ff pool co-resident with the
  rnn pool; was: ff DMA stalled the FF phase ~35us).
- j-granular DVE-add + tanh in the sweep loop (act_j overlaps the next
  j-group's matmuls; cuts the per-sweep critical path ~0.8us).
- decoder weights staged late: wd0 into the freed rnn space during FF,
  wdm/wdo after the ff pool closes (hidden behind ReduceScatter 0).
- collective payloads fp16 (half the wire bytes of f32).
- single rotating PSUM pool for FF/decoder accumulation groups.

See kernel_v2.py docstring for the algorithm description.
"""

import sys
import numpy as np

sys.path.insert(0, "/opt/trn_rl_repo")

import ml_dtypes

E = 16
L = 3
D_IN = 32
D = 512
H_FF = 2048
D_ENC = 512
N_DEC = 4
H_DEC = 2048
D_OUT = 1024
T_FULL = 128
N_CORES = 8

E_LOC = E // N_CORES
DT = D // 128
NFT = H_FF // 128
NCAT = (L * D) // 128
NHD = H_DEC // 128
KD0 = (E_LOC * D_ENC) // 128
HD_SH = H_DEC // N_CORES
KDM = HD_SH // 128
DO_SH = D_OUT // N_CORES

S_SWEEPS = 14

F16 = ml_dtypes.float16 if hasattr(ml_dtypes, "float16") else np.float16


def _tile_kxm(w):
    """[K, M] -> [128, nk*nm*128] with col ((i*nm)+j)*128 : lhsT tile (i,j)."""
    K, M = w.shape
    nk, nm = K // 128, M // 128
    return np.ascontiguousarray(
        w.reshape(nk, 128, nm, 128).transpose(1, 0, 2, 3).reshape(128, nk * nm * 128)
    )


def _bias_cols(b):
    """[M] -> [128, M//128] with col j holding b[j*128:(j+1)*128]."""
    return np.ascontiguousarray(b.reshape(-1, 128).T)


def build_nc(t_steps):
    from concourse import bacc, mybir, tile

    F32 = mybir.dt.float32
    FP16 = mybir.dt.float16
    AF = mybir.ActivationFunctionType
    ADD = mybir.AluOpType.add
    T = t_steps

    nc = bacc.Bacc(None, num_devices=N_CORES)

    # ---- I/O ----------------------------------------------------------------
    xT = nc.dram_tensor("xT", [D_IN, T], F32, kind="ExternalInput")
    win0 = [nc.dram_tensor(f"win0_{k}", [D_IN, D], F32, kind="ExternalInput")
            for k in range(E_LOC)]
    wh = [nc.dram_tensor(f"wh_{k}", [128, L * DT * DT * 128], FP16, kind="ExternalInput")
          for k in range(E_LOC)]
    win = [nc.dram_tensor(f"win_{k}", [128, (L - 1) * DT * DT * 128], FP16,
                          kind="ExternalInput") for k in range(E_LOC)]
    b_rnn = [nc.dram_tensor(f"b_{k}", [128, L * DT], F32, kind="ExternalInput")
             for k in range(E_LOC)]
    wff1 = [nc.dram_tensor(f"wff1_{k}", [128, NCAT * NFT * 128], FP16, kind="ExternalInput")
            for k in range(E_LOC)]
    bff1 = [nc.dram_tensor(f"bff1_{k}", [128, NFT], F32, kind="ExternalInput")
            for k in range(E_LOC)]
    wff2 = [nc.dram_tensor(f"wff2_{k}", [128, NFT * DT * 128], FP16, kind="ExternalInput")
            for k in range(E_LOC)]
    bff2 = [nc.dram_tensor(f"bff2_{k}", [128, DT], F32, kind="ExternalInput")
            for k in range(E_LOC)]
    wd0 = nc.dram_tensor("wd0", [128, KD0 * NHD * 128], FP16, kind="ExternalInput")
    bd0r = nc.dram_tensor("bd0r", [1, NHD * 128], FP16, kind="ExternalInput")
    wdm = [nc.dram_tensor(f"wdm{m}", [128, KDM * NHD * 128], FP16, kind="ExternalInput")
           for m in range(N_DEC - 2)]
    bdmr = [nc.dram_tensor(f"bdmr{m}", [1, NHD * 128], FP16, kind="ExternalInput")
            for m in range(N_DEC - 2)]
    NDO = D_OUT // 128            # 8 output j-tiles
    ident = nc.dram_tensor("ident", [128, 128], FP16, kind="ExternalInput")
    wdo = nc.dram_tensor("wdo", [128, KDM * NDO * 128], FP16, kind="ExternalInput")
    bdor = nc.dram_tensor("bdor", [1, NDO * 128], FP16, kind="ExternalInput")
    # per-core output-layer PARTIAL; the host sums the 8 cores' tensors
    y_out = nc.dram_tensor("y_out", [D_OUT, T], F32, kind="ExternalOutput")

    cc_in = [nc.dram_tensor(f"cc_in{m}", [H_DEC, T], FP16) for m in range(N_DEC - 1)]
    rs_out = [nc.dram_tensor(f"rs_out{m}", [HD_SH, T], FP16) for m in range(N_DEC - 1)]

    RG = [list(range(N_CORES))]

    def colw(i, j, nm):
        return (i * nm + j) * 128

    LCH = DT * DT * 128

    with tile.TileContext(nc, num_cores=N_CORES) as tc:
        with (
            tc.tile_pool(name="persist", bufs=1) as persist,
            tc.tile_pool(name="ps_main", bufs=4, space="PSUM") as ps_main,
            tc.tile_pool(name="tmp", bufs=4) as tmp_pool,
        ):
            # --- small persistent tensors
            xT_sb = persist.tile([D_IN, T], F32, name="xT", tag="xT")
            nc.sync.dma_start(xT_sb[:], xT[:])
            win0_sb, b_sb, bff1_sb, bff2_sb = [], [], [], []
            u_sb, ench_sb = [], []
            hh = [[None] * L for _ in range(E_LOC)]
            for k in range(E_LOC):
                w0 = persist.tile([D_IN, D], F32, name=f"win0_{k}", tag=f"win0_{k}")
                nc.sync.dma_start(w0[:], win0[k][:])
                win0_sb.append(w0)
                bb = persist.tile([128, L * DT], F32, name=f"b_{k}", tag=f"b_{k}")
                nc.sync.dma_start(bb[:], b_rnn[k][:])
                b_sb.append(bb)
                b1 = persist.tile([128, NFT], F32, name=f"bff1_{k}", tag=f"bff1_{k}")
                nc.sync.dma_start(b1[:], bff1[k][:])
                bff1_sb.append(b1)
                b2 = persist.tile([128, DT], F32, name=f"bff2_{k}", tag=f"bff2_{k}")
                nc.sync.dma_start(b2[:], bff2[k][:])
                bff2_sb.append(b2)
                u_sb.append(persist.tile([128, DT, T], FP16, name=f"u_{k}", tag=f"u_{k}"))
                ench_sb.append(persist.tile([128, DT, T], FP16, name=f"enc_{k}", tag=f"enc_{k}"))
                for l in range(L):
                    hh[k][l] = [
                        persist.tile([128, DT, 1 + T], FP16, name=f"hh_{k}_{l}_{b}",
                                     tag=f"hh_{k}_{l}_{b}")
                        for b in range(2)
                    ]
                    nc.vector.memset(hh[k][l][0][:], 0.0)
                    nc.vector.memset(hh[k][l][1][:, :, 0:1], 0.0)
            ident_sb = persist.tile([128, 128], FP16, name="ident", tag="ident")
            nc.sync.dma_start(ident_sb[:], ident[:])
            ones_sb = persist.tile([1, T], FP16, name="ones", tag="ones")
            nc.vector.memset(ones_sb[:], 1.0)
            bd0r_sb = persist.tile([1, NHD * 128], FP16, name="bd0r", tag="bd0r")
            nc.sync.dma_start(bd0r_sb[:], bd0r[:])

            with tc.tile_pool(name="ff", bufs=1) as ff:
                wff1_sb, wff2_sb, ffs_sb = [], [], []
                ffacc_sb = []
                for k in range(E_LOC):
                    wff1_sb.append(ff.tile([128, NCAT * NFT * 128], FP16,
                                           name=f"wff1_{k}", tag=f"wff1_{k}"))
                    wff2_sb.append(ff.tile([128, NFT * DT * 128], FP16,
                                           name=f"wff2_{k}", tag=f"wff2_{k}"))
                    ffacc_sb.append(ff.tile([128, NFT, T], FP16,
                                            name=f"ffacc_{k}", tag=f"ffacc_{k}"))

                with (
                    tc.tile_pool(name="rnn", bufs=1) as rnn,
                    tc.tile_pool(name="ps_sw", bufs=2, space="PSUM") as ps_sw,
                ):
                    wh_sb, win_sb = [], []
                    for k in range(E_LOC):
                        wh_sb.append(rnn.tile([128, L * LCH], FP16, name=f"wh_{k}", tag=f"wh_{k}"))
                        win_sb.append(rnn.tile([128, (L - 1) * LCH], FP16, name=f"win_{k}",
                                               tag=f"win_{k}"))
                    for l in range(L):
                        for k in range(E_LOC):
                            nc.sync.dma_start(wh_sb[k][:, l * LCH:(l + 1) * LCH],
                                              wh[k][:, l * LCH:(l + 1) * LCH])
                            if l < L - 1:
                                nc.sync.dma_start(win_sb[k][:, l * LCH:(l + 1) * LCH],
                                                  win[k][:, l * LCH:(l + 1) * LCH])
                    # ff weights stream in during the sweeps
                    for k in range(E_LOC):
                        half = NCAT * NFT * 128 // 2
                        nc.sync.dma_start(wff1_sb[k][:, 0:half], wff1[k][:, 0:half])
                        nc.sync.dma_start(wff1_sb[k][:, half:], wff1[k][:, half:])
                        nc.sync.dma_start(wff2_sb[k][:], wff2[k][:])

                    hfin = [[None] * E_LOC for _ in range(L)]
                    for l in range(L):
                        for k in range(E_LOC):
                            psu = ps_sw.tile([128, DT, T], F32, name=f"ps{k}", tag=f"ps{k}")
                            if l == 0:
                                for j in range(DT):
                                    nc.tensor.matmul(psu[:, j, :],
                                                     win0_sb[k][:, j * 128:(j + 1) * 128],
                                                     xT_sb[:], start=True, stop=True)
                            else:
                                hprev = hfin[l - 1][k]
                                for j in range(DT):
                                    for i in range(DT):
                                        nc.tensor.matmul(
                                            psu[:, j, :],
                                            win_sb[k][:, colw((l - 1) * DT + i, j, DT):
                                                      colw((l - 1) * DT + i, j, DT) + 128],
                                            hprev[:, i, 1:1 + T],
                                            start=(i == 0), stop=(i == DT - 1))
                            for j in range(DT):
                                nc.vector.tensor_scalar_add(
                                    u_sb[k][:, j, :], psu[:, j, :],
                                    b_sb[k][:, l * DT + j:l * DT + j + 1])

                        for s in range(S_SWEEPS):
                            korder = range(E_LOC) if s % 2 == 0 else range(E_LOC - 1, -1, -1)
                            for k in korder:
                                src = hh[k][l][s % 2]
                                dst = hh[k][l][(s + 1) % 2]
                                if s == 0:
                                    # zero state: sweep 0 is just tanh(u)
                                    nc.scalar.activation(dst[:, :, 1:1 + T], u_sb[k][:],
                                                         AF.Tanh)
                                    continue
                                ps = ps_sw.tile([128, DT, T], F32, name=f"ps{k}", tag=f"ps{k}")
                                for jp in range(DT // 2):
                                    for j in (2 * jp, 2 * jp + 1):
                                        if k == 1:
                                            # u injected via identity matmul;
                                            # tanh then reads PSUM directly
                                            nc.tensor.matmul(ps[:, j, :], ident_sb[:],
                                                             u_sb[k][:, j, :],
                                                             start=True, stop=False)
                                        for i in range(DT):
                                            nc.tensor.matmul(
                                                ps[:, j, :],
                                                wh_sb[k][:, colw(l * DT + i, j, DT):
                                                         colw(l * DT + i, j, DT) + 128],
                                                src[:, i, 0:T],
                                                start=(i == 0 and k != 1),
                                                stop=(i == DT - 1))
                                    if k == 1:
                                        nc.scalar.activation(dst[:, 2 * jp:2 * jp + 2, 1:1 + T],
                                                             ps[:, 2 * jp:2 * jp + 2, :],
                                                             AF.Tanh)
                                    else:
                                        tt = tmp_pool.tile([128, 2, T], FP16, name=f"tt{k}",
                                                           tag=f"tt{k}")
                                        nc.vector.tensor_add(tt[:], ps[:, 2 * jp:2 * jp + 2, :],
                                                             u_sb[k][:, 2 * jp:2 * jp + 2, :])
                                        nc.scalar.activation(dst[:, 2 * jp:2 * jp + 2, 1:1 + T],
                                                             tt[:], AF.Tanh)
                        for k in range(E_LOC):
                            hfin[l][k] = hh[k][l][S_SWEEPS % 2]

                # --- FF head + decoder layer-0 partial (rnn space now free) --
                with tc.tile_pool(name="dec_w", bufs=1) as dec_w:
                    ffs_sb = ffacc_sb   # gelu output overwrites the staging buffer
                    pd_sb = dec_w.tile([128, NHD, T], FP16, name="pd_sb", tag="pd_sb")
                    wd0_sb = dec_w.tile([128, KD0 * NHD * 128], FP16, name="wd0", tag="wd0")
                    csz = KD0 * NHD * 128 // 4
                    for ch in range(4):
                        nc.sync.dma_start(wd0_sb[:, ch * csz:(ch + 1) * csz],
                                          wd0[:, ch * csz:(ch + 1) * csz])

                    for k in range(E_LOC):
                        for g in range(NFT // DT):          # 4 m-tiles per psum bank
                            pf = ps_main.tile([128, DT, T], F32, name="pm", tag="pm")
                            for mi in range(DT):
                                m = g * DT + mi
                                idx = 0
                                for l in range(L):
                                    for j in range(DT):
                                        nc.tensor.matmul(
                                            pf[:, mi, :],
                                            wff1_sb[k][:, colw(l * DT + j, m, NFT):
                                                       colw(l * DT + j, m, NFT) + 128],
                                            hfin[l][k][:, j, 1:1 + T],
                                            start=(idx == 0), stop=(idx == NCAT - 1))
                                        idx += 1
                                nc.scalar.activation(ffs_sb[k][:, m, :], pf[:, mi, :],
                                                     AF.Gelu_apprx_tanh,
                                                     bias=bff1_sb[k][:, m:m + 1])
                        pf2 = ps_main.tile([128, DT, T], F32, name="pm", tag="pm")
                        for j in range(DT):
                            for i in range(NFT):
                                nc.tensor.matmul(
                                    pf2[:, j, :],
                                    wff2_sb[k][:, colw(i, j, DT):colw(i, j, DT) + 128],
                                    ffs_sb[k][:, i, :],
                                    start=(i == 0), stop=(i == NFT - 1))
                            nc.vector.tensor_scalar_add(ench_sb[k][:, j, :], pf2[:, j, :],
                                                        bff2_sb[k][:, j:j + 1])

                    # decoder layer 0: k-sharded partial over this core's encoders
                    for g in range(NHD // DT):
                        pd = ps_main.tile([128, DT, T], F32, name="pm", tag="pm")
                        for ji in range(DT):
                            j2 = g * DT + ji
                            nc.tensor.matmul(pd[:, ji, :],
                                             bd0r_sb[:, j2 * 128:(j2 + 1) * 128],
                                             ones_sb[:], start=True, stop=False)
                            for i in range(KD0):
                                nc.tensor.matmul(
                                    pd[:, ji, :],
                                    wd0_sb[:, colw(i, j2, NHD):colw(i, j2, NHD) + 128],
                                    ench_sb[i // DT][:, i % DT, :],
                                    start=False, stop=(i == KD0 - 1))
                        nc.vector.tensor_copy(pd_sb[:, g * DT:(g + 1) * DT, :], pd[:])
                        nc.sync.dma_start(
                            cc_in[0][g * 512:(g + 1) * 512, :].rearrange(
                                "(i p) t -> p i t", p=128),
                            pd_sb[:, g * DT:(g + 1) * DT, :])
                    nc.gpsimd.collective_compute(
                        "ReduceScatter", ADD, replica_groups=RG,
                        ins=[cc_in[0][:]], outs=[rs_out[0][:]])

            # --- decoder mid/out (ff space now free) -------------------------
            with tc.tile_pool(name="dec2", bufs=1) as dec2:
                wdm_sb, bdmr_sb = [], []
                for m in range(N_DEC - 2):
                    t_ = dec2.tile([128, KDM * NHD * 128], FP16, name=f"wdm{m}", tag=f"wdm{m}")
                    nc.sync.dma_start(t_[:], wdm[m][:])
                    wdm_sb.append(t_)
                    t_ = dec2.tile([1, NHD * 128], FP16, name=f"bdmr{m}", tag=f"bdmr{m}")
                    nc.sync.dma_start(t_[:], bdmr[m][:])
                    bdmr_sb.append(t_)
                wdo_sb = dec2.tile([128, KDM * NDO * 128], FP16, name="wdo", tag="wdo")
                nc.sync.dma_start(wdo_sb[:], wdo[:])
                bdor_sb = dec2.tile([1, NDO * 128], FP16, name="bdor", tag="bdor")
                nc.sync.dma_start(bdor_sb[:], bdor[:])
                pd_sb = dec2.tile([128, NHD, T], FP16, name="pd_sb2", tag="pd_sb2")

                for m in range(N_DEC - 2):
                    zin = dec2.tile([128, KDM, T], FP16, name=f"zin{m}", tag=f"zin{m}")
                    zloc = dec2.tile([128, KDM, T], FP16, name=f"z{m}", tag=f"z{m}")
                    for i in range(KDM):
                        nc.sync.dma_start(
                            zin[:, i, :],
                            rs_out[m][i * 128:(i + 1) * 128, :].rearrange(
                                "(i p) t -> p i t", p=128))
                        nc.scalar.activation(zloc[:, i, :], zin[:, i, :], AF.Tanh)
                    pms = []
                    for g in range(NHD // DT):       # bias rows: no dep on the RS
                        pm = ps_main.tile([128, DT, T], F32, name="pm", tag="pm")
                        pms.append(pm)
                        for ji in range(DT):
                            j2 = g * DT + ji
                            nc.tensor.matmul(pm[:, ji, :],
                                             bdmr_sb[m][:, j2 * 128:(j2 + 1) * 128],
                                             ones_sb[:], start=True, stop=False)
                    for g in range(NHD // DT):
                        pm = pms[g]
                        for ji in range(DT):
                            j2 = g * DT + ji
                            for i in range(KDM):
                                nc.tensor.matmul(
                                    pm[:, ji, :],
                                    wdm_sb[m][:, colw(i, j2, NHD):colw(i, j2, NHD) + 128],
                                    zloc[:, i, :],
                                    start=False, stop=(i == KDM - 1))
                        nc.vector.tensor_copy(pd_sb[:, g * DT:(g + 1) * DT, :], pm[:])
                        nc.sync.dma_start(
                            cc_in[m + 1][g * 512:(g + 1) * 512, :].rearrange(
                                "(i p) t -> p i t", p=128),
                            pd_sb[:, g * DT:(g + 1) * DT, :])
                    nc.gpsimd.collective_compute(
                        "ReduceScatter", ADD, replica_groups=RG,
                        ins=[cc_in[m + 1][:]], outs=[rs_out[m + 1][:]])

                # output layer: k-sharded partial, ReduceScatter straight into y_out
                zin3 = dec2.tile([128, KDM, T], FP16, name="zin3", tag="zin3")
                z3 = dec2.tile([128, KDM, T], FP16, name="z3", tag="z3")
                for i in range(KDM):
                    nc.sync.dma_start(
                        zin3[:, i, :],
                        rs_out[N_DEC - 2][i * 128:(i + 1) * 128, :].rearrange(
                            "(i p) t -> p i t", p=128))
                    nc.scalar.activation(z3[:, i, :], zin3[:, i, :], AF.Tanh)
                yp_sb = dec2.tile([128, NDO, T], F32, name="yp_sb", tag="yp_sb")
                pys = []
                for g in range(NDO // DT):
                    py = ps_main.tile([128, DT, T], F32, name="pm", tag="pm")
                    pys.append(py)
                    for ji in range(DT):
                        j2 = g * DT + ji
                        nc.tensor.matmul(py[:, ji, :],
                                         bdor_sb[:, j2 * 128:(j2 + 1) * 128],
                                         ones_sb[:], start=True, stop=False)
                for g in range(NDO // DT):
                    py = pys[g]
                    for ji in range(DT):
                        j2 = g * DT + ji
                        for i in range(KDM):
                            nc.tensor.matmul(
                                py[:, ji, :],
                                wdo_sb[:, colw(i, j2, NDO):colw(i, j2, NDO) + 128],
                                z3[:, i, :],
                                start=False, stop=(i == KDM - 1))
                    nc.vector.tensor_copy(yp_sb[:, g * DT:(g + 1) * DT, :], py[:])
                    nc.sync.dma_start(
                        y_out[g * 512:(g + 1) * 512, :].rearrange("(i p) t -> p i t", p=128),
                        yp_sb[:, g * DT:(g + 1) * DT, :])

    nc.compile()
    return nc


def prep_inputs(inputs, t_steps):
    """Build the 8 per-core input maps from full numpy inputs."""
    T = t_steps
    f32 = lambda a: np.asarray(a, np.float32)
    x = f32(inputs["x"])
    W_in0, Wh0, b0 = f32(inputs["W_in0"]), f32(inputs["Wh0"]), f32(inputs["b0"])
    W_in_rest, Wh_rest, b_rest = (f32(inputs["W_in_rest"]), f32(inputs["Wh_rest"]),
                                  f32(inputs["b_rest"]))
    W_ff1, b_ff1 = f32(inputs["W_ff1"]), f32(inputs["b_ff1"])
    W_ff2, b_ff2 = f32(inputs["W_ff2"]), f32(inputs["b_ff2"])
    W_d0, b_d0 = f32(inputs["W_d0"]), f32(inputs["b_d0"])
    W_dmid, b_dmid = f32(inputs["W_dmid"]), f32(inputs["b_dmid"])
    W_dout, b_dout = f32(inputs["W_dout"]), f32(inputs["b_dout"])

    xT = np.ascontiguousarray(x[0, :T].T)
    in_maps = []
    for c in range(N_CORES):
        m = {"xT": xT}
        for k in range(E_LOC):
            e = E_LOC * c + k
            m[f"win0_{k}"] = np.ascontiguousarray(W_in0[e])
            wh_all = np.concatenate([Wh0[e][None], Wh_rest[e]], 0)
            m[f"wh_{k}"] = _tile_kxm(wh_all.reshape(L * D, D)).astype(F16)
            m[f"win_{k}"] = _tile_kxm(W_in_rest[e].reshape((L - 1) * D, D)).astype(F16)
            b_all = np.concatenate([b0[e][None], b_rest[e]], 0).reshape(-1)
            m[f"b_{k}"] = _bias_cols(b_all)
            m[f"wff1_{k}"] = _tile_kxm(W_ff1[e]).astype(F16)
            m[f"bff1_{k}"] = _bias_cols(b_ff1[e])
            m[f"wff2_{k}"] = _tile_kxm(W_ff2[e]).astype(F16)
            m[f"bff2_{k}"] = _bias_cols(b_ff2[e])
        m["wd0"] = _tile_kxm(W_d0[c * E_LOC * D_ENC:(c + 1) * E_LOC * D_ENC, :]).astype(F16)
        m["bd0r"] = np.ascontiguousarray((b_d0 / N_CORES)[None, :]).astype(F16)
        for mm in range(N_DEC - 2):
            m[f"wdm{mm}"] = _tile_kxm(W_dmid[mm][c * HD_SH:(c + 1) * HD_SH, :]).astype(F16)
            m[f"bdmr{mm}"] = np.ascontiguousarray((b_dmid[mm] / N_CORES)[None, :]).astype(F16)
        m["ident"] = np.eye(128, dtype=np.float32).astype(F16)
        m["wdo"] = _tile_kxm(W_dout[c * HD_SH:(c + 1) * HD_SH, :]).astype(F16)
        m["bdor"] = np.ascontiguousarray((b_dout / N_CORES)[None, :]).astype(F16)
        in_maps.append(m)
    return in_maps


def run(inputs, t_steps=T_FULL, trace=False):
    from concourse.bass_utils import run_bass_kernel_spmd

    nc = build_nc(t_steps)
    in_maps = prep_inputs(inputs, t_steps)
    res = run_bass_kernel_spmd(nc, in_maps, list(range(N_CORES)), trace=trace)
    acc = np.zeros((D_OUT, t_steps), np.float32)
    for c in range(N_CORES):
        acc += np.asarray(res.results[c]["y_out"], np.float32)
    return acc.T[None], res


def kernel(**inputs):
    y, _ = run(inputs, T_FULL, trace=False)
    return y
